# revision 22
# baseline (speedup 1.0000x reference)
"""Trainium2 Bass kernel for nn_BondHead2 (dense_mlp), v3.

Per batch element b (8, one per NeuronCore):
    h = LN(gelu(x @ W1 + b1)); h = LN(gelu(h @ W2 + b2)); h = LN(gelu(h @ W3 + b3))
    out = h @ Wo + bo;  out = (out + out^T_{seq axes}) / 2

v3 vs v2 (all targeting the TimelineSim cost model):
  - stats batches of 8 tiles (18 batches); the vocab projection rides the
    layer-2 mean-stats matmuls for free (stat rows 0:32 + proj rows 32:112
    share one PSUM bank; matmul cost is charged per output column only).
  - gelu / square / h-mul in [128, 2048] tiles: half the Act/DVE
    instruction count of v2.
  - L0 of batch s and L1 of batch s-2 share one stats bank (64 rows) ->
    one Act drain + one fat DMA + one rd write + one repl broadcast DMA
    per step covers both layers' rstd chains.
  - DMA consolidation: the rstd broadcast is ONE DMA per step (was 8 per
    batch-layer), mobuf write one DMA per L2 batch, fat reshape one DMA.
  - unit-level software pipeline (stats lag mains by one unit, crossing
    step boundaries) so the PE never waits on Act gelu or DVE square.
  - pipeline: L0(b)@step b, L1(b)@b+2, L2(b)@b+4, drains @b+5; both
    apply_h consumers of a step's repl run exactly 2 steps later.
"""

import numpy as np

import concourse.bacc as bacc
import concourse.bass as bass
import concourse.mybir as mybir
import concourse.tile as tile
from concourse.bass_utils import run_bass_kernel_spmd

F16 = mybir.dt.float16
F32 = mybir.dt.float32
U32 = mybir.dt.uint32
AF = mybir.ActivationFunctionType
OP = mybir.AluOpType

H = 64            # hidden dim
S = 384           # seq
T = S * S         # tokens per core (147456)
V = 5             # vocab
N = 512           # tokens per matmul tile (free dim)
SB = 8            # tiles per stats batch
NB = T // (2 * SB * N)   # 18 batches
BOFF = SB * N     # batch-local offset of group B (4096)
BLK = 2 * BOFF    # tokens per batch (8192)
SBLK = 2 * BLK    # tokens per sym block (16384; block k = batches 2k, 2k+1)
EPS = 1e-5
MAGIC = 0x5F3759DF

_CACHE: dict = {}


def _build_nc():
    nc = bacc.Bacc("TRN2", target_bir_lowering=False, debug=False)

    # ---- external inputs ----
    xf = nc.dram_tensor("xf", (H, T), F16, kind="ExternalInput").ap()
    wmain = nc.dram_tensor("wmain", (128, 3 * 128), F16, kind="ExternalInput").ap()
    # per bt: cols 0:32 g-selector (means), cols 32:64 s-selector (msqs)
    wstat = nc.dram_tensor("wstat", (128, SB * 64), F16, kind="ExternalInput").ap()
    # L2 merged: per bt [128, 112]: stats-mean cols 0:32 + proj cols 32:112
    wsp = nc.dram_tensor("wsp", (128, SB * 112), F16, kind="ExternalInput").ap()
    biases = nc.dram_tensor("biases", (128, 3), F32, kind="ExternalInput").ap()
    id128 = nc.dram_tensor("id128", (128, 128), F16, kind="ExternalInput").ap()

    # ---- internal DRAM ----
    mobuf = nc.dram_tensor("mobuf", (V * T,), F16)          # [v, tok] linear
    r3buf = nc.dram_tensor("r3buf", (T,), F16)              # [tok] linear
    rdbuf = nc.dram_tensor("rdbuf", (NB + 2, 2 * BLK), F16)  # rstd per step

    out_vm = nc.dram_tensor("out_vm", (V * T,), F16, kind="ExternalOutput").ap()

    with tile.TileContext(nc) as tc:
        with tc.tile_pool(name="wpool", bufs=1) as wpool:
            from contextlib import ExitStack
            mstack = ExitStack()
            xpool = mstack.enter_context(tc.tile_pool(name="xpool", bufs=3))
            gpool = mstack.enter_context(tc.tile_pool(name="gpool", bufs=18))
            spool = mstack.enter_context(tc.tile_pool(name="spool", bufs=4))
            fpool = mstack.enter_context(tc.tile_pool(name="fpool", bufs=2))
            rpool = mstack.enter_context(tc.tile_pool(name="rpool", bufs=3))
            mps = mstack.enter_context(
                tc.tile_pool(name="mps", bufs=1, space="PSUM"))
            stps = mstack.enter_context(
                tc.tile_pool(name="stps", bufs=2, space="PSUM"))
            st2ps = mstack.enter_context(
                tc.tile_pool(name="st2ps", bufs=1, space="PSUM"))
            syps = mstack.enter_context(
                tc.tile_pool(name="syps", bufs=1, space="PSUM"))
            sy = mstack.enter_context(tc.tile_pool(name="sypool", bufs=3))

            # resident weights (wmain first: the first mains need only wm
            # and the first x chunk, so keep the DMA queue short up front)
            wm = wpool.tile([128, 3 * 128], F16)
            nc.sync.dma_start(out=wm[:], in_=wmain)
            bcol = wpool.tile([128, 3], F32)
            nc.sync.dma_start(out=bcol[:], in_=biases)
            ws = wpool.tile([128, SB * 64], F16)
            wp2 = wpool.tile([128, SB * 112], F16)
            magic = wpool.tile([128, 1], U32)
            nc.vector.memset(magic[:], MAGIC)
            oneu = wpool.tile([128, 1], U32)
            nc.vector.memset(oneu[:], 1)
            chalf = wpool.tile([128, 1], F32)
            nc.vector.memset(chalf[:], -0.5)
            c15 = wpool.tile([128, 1], F32)
            nc.vector.memset(c15[:], 1.5)
            ceps = wpool.tile([128, 1], F32)
            nc.vector.memset(ceps[:], EPS)
            idt = wpool.tile([128, 128], F16)

            gstate = {}    # (b, layer) -> [g_h0, g_h1] tiles [128, 2048]
            xstate = {}    # b -> ("split"|"whole", tiles)
            bankst = {}    # step -> paired L0/L1 stats bank [64, N]
            bank2 = {}     # b -> L2 stats+proj bank [112, N]
            replst = {}    # step -> repl tile [128, 2*BOFF]
            sunits = {}    # (b, layer, half) -> s tile
            pend = []      # stats units pending (lag 1 behind mains)

            def rsqrt_to(v_f32, out_ap, sh, tg):
                """out <- rsqrt(v) via quake seed + 1 Newton step.

                Seed bit ops on DVE (hw GPSIMD lacks 32-bit shift); the
                Newton polynomial runs on the otherwise idle Pool engine.
                """
                y32 = fpool.tile(list(sh), F32, tag="nry" + tg)
                yi = y32[:].bitcast(U32)
                vi = v_f32.bitcast(U32)
                nc.vector.tensor_tensor(
                    yi, vi, oneu[:].to_broadcast(sh), OP.logical_shift_right)
                nc.vector.tensor_tensor(
                    yi, magic[:].to_broadcast(sh), yi, OP.subtract)
                t = fpool.tile(list(sh), F32, tag="nrt" + tg)
                nc.gpsimd.tensor_mul(t[:], y32[:], y32[:])
                nc.gpsimd.tensor_mul(t[:], t[:], v_f32)
                nc.gpsimd.tensor_mul(t[:], t[:], chalf[:].to_broadcast(sh))
                nc.gpsimd.tensor_tensor(
                    t[:], t[:], c15[:].to_broadcast(sh), OP.add)
                nc.gpsimd.tensor_mul(out_ap, t[:], y32[:])

            def prefetch_x(b, split=False):
                if split:
                    xch = []
                    for c in range(4):
                        xt = xpool.tile([128, 1024], F16, tag="xs", bufs=4)
                        src = bass.AP(
                            tensor=xf.tensor,
                            offset=b * BLK + c * 1024,
                            ap=[[BOFF, 2], [T, 64], [1, 1024]],
                        )
                        nc.sync.dma_start(out=xt[:], in_=src)
                        xch.append(xt)
                    xstate[b] = ("split", xch)
                else:
                    xt = xpool.tile([128, BOFF], F16, tag="x")
                    src = bass.AP(
                        tensor=xf.tensor,
                        offset=b * BLK,
                        ap=[[BOFF, 2], [T, 64], [1, BOFF]],
                    )
                    nc.sync.dma_start(out=xt[:], in_=src)
                    xstate[b] = ("whole", xt)

            def x_slice(b, bt):
                kind, xv = xstate[b]
                if kind == "split":
                    return xv[bt // 2][:, (bt % 2) * N:(bt % 2 + 1) * N]
                return xv[:, bt * N:(bt + 1) * N]

            # ---------------- unit machinery ----------------

            def mains(b, layer, half):
                mpair = mps.tile([128, 4, N], F32, tag="m")
                for k in range(4):
                    bt = 4 * half + k
                    if layer == 0:
                        rhs = x_slice(b, bt)
                    else:
                        gh = gstate[(b, layer - 1)][bt // 4]
                        rhs = gh[:, (bt % 4) * N:(bt % 4 + 1) * N]
                    nc.tensor.matmul(
                        mpair[:, k, :],
                        wm[:, 128 * layer:128 * (layer + 1)],
                        rhs, start=True, stop=True,
                    )
                if layer > 0 and half == 1:
                    gstate.pop((b, layer - 1))
                if layer == 0 and half == 1:
                    xstate.pop(b, None)
                g = gpool.tile([128, 4 * N], F16, tag="g")
                # two gelus: banks 0-1 free as soon as the first finishes,
                # so the next unit's mains are never gated on the full tile
                nc.scalar.activation(
                    g[:, 0:2 * N],
                    mpair[:, 0:2, :].rearrange("p a n -> p (a n)"),
                    AF.Gelu, bias=bcol[:, layer:layer + 1], scale=1.0,
                )
                nc.scalar.activation(
                    g[:, 2 * N:4 * N],
                    mpair[:, 2:4, :].rearrange("p a n -> p (a n)"),
                    AF.Gelu, bias=bcol[:, layer:layer + 1], scale=1.0,
                )
                s = spool.tile([128, 4 * N], F16, tag="s", bufs=4)
                nc.vector.tensor_mul(s[:], g[:], g[:])
                gstate.setdefault((b, layer), [None, None])[half] = g
                sunits[(b, layer, half)] = s

            def stats(b, layer, half):
                """8 matmuls: mean(+proj for L2) on g, msq on s."""
                g = gstate[(b, layer)][half]
                s = sunits.pop((b, layer, half))
                if layer == 2:
                    bank = bank2[b]
                    for k in range(4):
                        bt = 4 * half + k
                        nc.tensor.matmul(
                            bank[0:112], wp2[:, 112 * bt:112 * bt + 112],
                            g[:, k * N:(k + 1) * N],
                            start=(half == 0 and k == 0), stop=False,
                            skip_group_check=True,
                        )
                        nc.tensor.matmul(
                            bank[0:32], ws[:, 64 * bt + 32:64 * bt + 64],
                            s[:, k * N:(k + 1) * N],
                            start=False, stop=(half == 1 and k == 3),
                            skip_group_check=True,
                        )
                else:
                    # L0 of batch b shares bank[64, N] with L1 of batch b-2
                    # (same step): L0 rows 0:32, L1 rows 32:64.  Each 32-row
                    # region is its own accumulation group (start=True on
                    # its first matmul resets only that region).
                    step_key = b + 2 * layer
                    bank = bankst[step_key]
                    for k in range(4):
                        bt = 4 * half + k
                        nc.tensor.matmul(
                            bank[32 * layer:32 * layer + 32],
                            ws[:, 64 * bt:64 * bt + 32],
                            g[:, k * N:(k + 1) * N],
                            start=(half == 0 and k == 0), stop=False,
                            skip_group_check=True,
                        )
                        nc.tensor.matmul(
                            bank[32 * layer:32 * layer + 32],
                            ws[:, 64 * bt + 32:64 * bt + 64],
                            s[:, k * N:(k + 1) * N],
                            start=False, stop=(half == 1 and k == 3),
                            skip_group_check=True,
                        )

            # ---------------- stats postprocessing ----------------
            # Bank row layout per 32-row region: 0:8 grpA-mean(bt), 8:16
            # grpB-mean, 16:24 grpA-msq, 24:32 grpB-msq; bank col = c (512).
            # fat layout: partition p = 64l + 32g + 4bt + c//128, giving a
            # FLAT rf index = group-local token, so the rd write is linear.

            def drain_pair(step):
                bank = bankst.pop(step)
                rowb = fpool.tile([64, N], F16, tag="rowb")
                nc.scalar.copy(rowb[:], bank[:])
                # mean rows 32l + 8g + bt (msq rows between the l blocks)
                # -> meanfat flat index = l*BLK + g*BOFF + bt*N + c
                meanfat = fpool.tile([128, 128], F16, tag="meanfat")
                msqfat = fpool.tile([128, 128], F16, tag="msqfat")
                for l in range(2):
                    nc.sync.dma_start(
                        out=meanfat[64 * l:64 * (l + 1), :],
                        in_=rowb[32 * l:32 * l + 16, :]
                        .rearrange("p (q c) -> p q c", q=4),
                    )
                    nc.sync.dma_start(
                        out=msqfat[64 * l:64 * (l + 1), :],
                        in_=rowb[32 * l + 16:32 * l + 32, :]
                        .rearrange("p (q c) -> p q c", q=4),
                    )
                # var = msq - mean^2 + eps  (fp32, on Pool)
                sqf = fpool.tile([128, 128], F32, tag="sqf")
                nc.gpsimd.tensor_mul(sqf[:], meanfat[:], meanfat[:])
                varf = fpool.tile([128, 128], F32, tag="varf")
                nc.gpsimd.tensor_tensor(
                    varf[:], msqfat[:], sqf[:], OP.subtract)
                nc.gpsimd.tensor_tensor(
                    varf[:], varf[:], ceps[:].to_broadcast((128, 128)), OP.add)
                rf = fpool.tile([128, 128], F16, tag="rf")
                rsqrt_to(varf[:], rf[:], (128, 128), "p")
                # rf flat index == l*BLK + g*BOFF + bt*N + c  -> linear write
                nc.sync.dma_start(
                    out=bass.AP(tensor=rdbuf, offset=step * 2 * BLK,
                                ap=[[128, 128], [1, 128]]),
                    in_=rf[:],
                )
                repl = rpool.tile([128, 2 * BOFF], F16, tag="repl")
                for l in range(2):
                    src2 = bass.AP(
                        tensor=rdbuf, offset=step * 2 * BLK + l * BLK,
                        # dims (grp, dup64, c)
                        ap=[[BOFF, 2], [0, 64], [1, BOFF]],
                    )
                    nc.sync.dma_start(
                        out=repl[:, l * BOFF:(l + 1) * BOFF], in_=src2)
                replst[step] = repl

            def apply_h(step, b, layer):
                """h = g * rstd in place (repl cols: L0 0:BOFF, L1 BOFF:)."""
                repl = replst[step]
                off = layer * BOFF
                for half in range(2):
                    g = gstate[(b, layer)][half]
                    nc.vector.tensor_mul(
                        g[:], g[:],
                        repl[:, off + half * 4 * N: off + (half + 1) * 4 * N])

            def drain_l2(b):
                """L2 bank: stats rows 0:32 -> r3buf; proj rows 32:112 -> mobuf."""
                bank = bank2.pop(b)
                dr = fpool.tile([112, N], F16, tag="dr2", bufs=3)
                nc.scalar.copy(dr[:], bank[:])
                gstate.pop((b, 2), None)
                # proj rows 32:112 are (g, v, bt)-ordered: plain read, the
                # structure lives on the DRAM side: dims (g, v, bt, c)
                for g2 in range(2):
                    dst = bass.AP(
                        tensor=mobuf, offset=b * BLK + g2 * BOFF,
                        ap=[[T, V], [1, BOFF]],
                    )
                    nc.sync.dma_start(
                        out=dst, in_=dr[32 + 40 * g2:72 + 40 * g2, :])
                # stats: mean rows 0:16 / msq rows 16:32, both linear in
                # (g, bt, c) -> [128, 64] fats with flat index = token
                mean2 = fpool.tile([128, 64], F16, tag="mean2")
                nc.sync.dma_start(
                    out=mean2[:],
                    in_=dr[0:16, :].rearrange("p (q c) -> p q c", q=8),
                )
                msq2 = fpool.tile([128, 64], F16, tag="msq2")
                nc.sync.dma_start(
                    out=msq2[:],
                    in_=dr[16:32, :].rearrange("p (q c) -> p q c", q=8),
                )
                sqf = fpool.tile([128, 64], F32, tag="sqf2")
                nc.gpsimd.tensor_mul(sqf[:], mean2[:], mean2[:])
                varf = fpool.tile([128, 64], F32, tag="varf2")
                nc.gpsimd.tensor_tensor(
                    varf[:], msq2[:], sqf[:], OP.subtract)
                nc.gpsimd.tensor_tensor(
                    varf[:], varf[:], ceps[:].to_broadcast((128, 64)), OP.add)
                rf3 = fpool.tile([128, 64], F16, tag="rf3")
                rsqrt_to(varf[:], rf3[:], (128, 64), "3")
                # rf3 flat index == group-local token -> linear write
                nc.sync.dma_start(
                    out=bass.AP(tensor=r3buf, offset=b * BLK,
                                ap=[[64, 128], [1, 64]]),
                    in_=rf3[:],
                )

            # ---------------- symmetrization ----------------
            sym_pmap = {}

            def sym_prep(blk):
                mo = sy.tile([128, V, 128], F16, tag="mo_in", bufs=2)
                src = bass.AP(
                    tensor=mobuf, offset=blk * SBLK,
                    ap=[[128, 128], [T, V], [1, 128]],
                )
                nc.sync.dma_start(out=mo[:], in_=src)
                r = sy.tile([128, 128], F16, tag="r_in", bufs=2)
                rsrc = bass.AP(
                    tensor=r3buf, offset=blk * SBLK,
                    ap=[[128, 128], [1, 128]],
                )
                nc.sync.dma_start(out=r[:], in_=rsrc)
                p_ = sy.tile([128, V, 128], F16, tag="p", bufs=5)
                ra = r[:]
                rbc = bass.AP(tensor=ra.tensor, offset=ra.offset,
                              ap=[ra.ap[0], [0, V], ra.ap[1]])
                nc.vector.tensor_mul(p_[:], mo[:], rbc)
                sym_pmap[blk] = p_

            def sym_transposes(p_):
                pt = syps.tile([128, V, 128], F16, tag="pt")
                for v in range(V):
                    nc.tensor.transpose(pt[:, v, :], p_[:, v, :], idt[:])
                return pt

            def sym_emit(pa, pt, bi, bj):
                o = sy.tile([128, V, 128], F16, tag="o", bufs=2)
                nc.vector.tensor_add(
                    o[:].rearrange("p a n -> p (a n)"),
                    pa[:].rearrange("p a n -> p (a n)"),
                    pt[:].rearrange("p a n -> p (a n)"),
                )
                d1 = bass.AP(
                    tensor=out_vm.tensor, offset=bi * 128 * S + bj * 128,
                    ap=[[S, 128], [T, V], [1, 128]],
                )
                nc.sync.dma_start(out=d1, in_=o[:])

            def sym_pair(bi, bj):
                pa = sym_pmap.pop(3 * bi + bj)
                if bi == bj:
                    sym_emit(pa, sym_transposes(pa), bi, bj)
                else:
                    pb = sym_pmap.pop(3 * bj + bi)
                    sym_emit(pa, sym_transposes(pb), bi, bj)
                    sym_emit(pb, sym_transposes(pa), bj, bi)

            # sym block k = batches 2k, 2k+1; r3/mobuf of batch b land in
            # step b+4, so block k is ready at step 2k+5.
            sym_sched = {}
            sym_prep_sched = {}
            for bi in range(3):
                for bj in range(bi + 1):
                    gate = 2 * max(3 * bi + bj, 3 * bj + bi) + 5
                    sym_sched.setdefault(gate, []).append((bi, bj))
                    for blk in {3 * bi + bj, 3 * bj + bi}:
                        sym_prep_sched.setdefault(gate, []).append(blk)

            prefetch_x(0, split=True)
            nc.sync.dma_start(out=ws[:], in_=wstat)
            nc.sync.dma_start(out=wp2[:], in_=wsp)
            nc.sync.dma_start(out=idt[:], in_=id128)

            # pipeline: L0(b)@b, L1(b)@b+2, L2(b)@b+3; drain_l2(b)@b+4
            # (early in the step, so the freed bank never stalls this
            # step's L2 stats); stats lag mains by one unit crossing step
            # boundaries; apply_h(L0(b)) runs in step b+1's DVE tail,
            # apply_h(L1(b)) early in step b+3 (interleaved after unit 1
            # so it never heads-of-line-blocks the squares).
            def emit_step(step):
                units = []
                if step < NB:
                    units.append((step, 0, 0))
                    units.append((step, 0, 1))
                if 0 <= step - 2 < NB:
                    units.append((step - 2, 1, 0))
                    units.append((step - 2, 1, 1))
                if 0 <= step - 3 < NB:
                    units.append((step - 3, 2, 0))
                    units.append((step - 3, 2, 1))

                has_l0 = step < NB
                has_l1 = 0 <= step - 2 < NB
                if has_l0 or has_l1:
                    bk = stps.tile([64, N], F32, tag="stat", name="bk")
                    bankst[step] = bk
                    if not has_l0:
                        nc.vector.memset(bk[0:32], 0.0)
                    if not has_l1:
                        nc.vector.memset(bk[32:64], 0.0)
                if 0 <= step - 3 < NB:
                    bank2[step - 3] = st2ps.tile([112, N], F32, tag="st2",
                                                 name="bk2")

                if step + 1 < NB:
                    prefetch_x(step + 1)

                def unit(i):
                    mains(*units[i])
                    if pend:
                        stats(*pend.pop(0))
                    pend.append(units[i])

                applied_l1 = False

                def apply_l1():
                    # h for this step's L2 mains: repl(step-1)
                    if 0 <= step - 3 < NB:
                        apply_h(step - 1, step - 3, 1)

                if units:
                    if units[0][1] == 2:
                        apply_l1()
                        applied_l1 = True
                    unit(0)
                else:
                    while pend:
                        stats(*pend.pop(0))
                # L2h1(step-4)'s stats were just flushed -> its bank drains
                # now, freeing st2ps before this step's L2 stats need it
                if 0 <= step - 4 < NB:
                    drain_l2(step - 4)
                for i in range(1, len(units)):
                    if units[i][1] == 2 and not applied_l1:
                        apply_l1()
                        applied_l1 = True
                    unit(i)
                if not applied_l1:
                    apply_l1()
                # flush any pending unit whose bank drains this step (only
                # happens in warmup/tail steps; steady state keeps the lag)
                while pend and pend[0][0] + (0, 2, 4)[pend[0][1]] <= step:
                    stats(*pend.pop(0))

                if step in bankst:
                    drain_pair(step)

                for blk in sym_prep_sched.get(step, []):
                    sym_prep(blk)
                for (bi, bj) in sym_sched.get(step, []):
                    sym_pair(bi, bj)

                # h for next step's L1 mains, in this step's DVE tail
                if 0 <= step - 1 < NB:
                    apply_h(step - 1, step - 1, 0)
                replst.pop(step - 1, None)

            for step in range(NB + 4):
                emit_step(step)
            mstack.close()

    nc.compile()
    return nc


def _prep_weights(inputs):
    W1 = np.asarray(inputs["W1"], np.float64)
    W2 = np.asarray(inputs["W2"], np.float64)
    W3 = np.asarray(inputs["W3"], np.float64)
    Wo = np.asarray(inputs["Wo"], np.float64)
    b1 = np.asarray(inputs["b1"], np.float64)
    b2 = np.asarray(inputs["b2"], np.float64)
    b3 = np.asarray(inputs["b3"], np.float64)
    bo = np.asarray(inputs["bo"], np.float64)
    ln_g = np.asarray(inputs["ln_g"], np.float64)
    ln_b = np.asarray(inputs["ln_b"], np.float64)

    C = np.eye(H) - np.ones((H, H)) / H
    F = C @ np.diag(ln_g)
    Ws = [W1, F @ W2, F @ W3]
    bs = [b1, b2 + W2.T @ ln_b, b3 + W3.T @ ln_b]
    Woh = 0.5 * (F @ Wo)
    boh = (bo + Wo.T @ ln_b).astype(np.float32)

    wmain = np.zeros((128, 3 * 128), np.float16)
    for l, W in enumerate(Ws):
        wmain[0:64, 128 * l:128 * l + 64] = W.astype(np.float16)
        wmain[64:128, 128 * l + 64:128 * l + 128] = W.astype(np.float16)

    # per bt: g-selector cols 0:32 (rows 0:8 A-mean(bt), 8:16 B-mean),
    # s-selector cols 32:64 (rows 16:24 A-msq, 24:32 B-msq)
    wstat = np.zeros((128, SB * 64), np.float16)
    for bt in range(SB):
        wstat[0:64, 64 * bt + bt] = np.float16(1 / 64)
        wstat[64:128, 64 * bt + 8 + bt] = np.float16(1 / 64)
        wstat[0:64, 64 * bt + 32 + 16 + bt] = np.float16(1 / 64)
        wstat[64:128, 64 * bt + 32 + 24 + bt] = np.float16(1 / 64)

    # L2 merged: per bt [128, 112]: mean cols 0:32 + proj rows
    # 32 + 40g + 8v + bt
    wsp = np.zeros((128, SB * 112), np.float16)
    w16 = Woh.astype(np.float16)
    for bt in range(SB):
        wsp[0:64, 112 * bt + bt] = np.float16(1 / 64)
        wsp[64:128, 112 * bt + 8 + bt] = np.float16(1 / 64)
        for v in range(V):
            wsp[0:64, 112 * bt + 32 + 8 * v + bt] = w16[:, v]
            wsp[64:128, 112 * bt + 32 + 40 + 8 * v + bt] = w16[:, v]

    biases = np.zeros((128, 3), np.float32)
    for l, bb in enumerate(bs):
        biases[0:64, l] = bb.astype(np.float32)
        biases[64:128, l] = bb.astype(np.float32)
    id128 = np.eye(128, dtype=np.float16)
    return dict(wmain=wmain, wstat=wstat, wsp=wsp, biases=biases,
                id128=id128), boh


def _prep_x(xb):
    """[S, S, H] fp32 -> [H, T] fp16 in block-major token order."""
    t = xb.reshape(3, 128, 3, 128, H).transpose(0, 2, 1, 3, 4).reshape(T, H)
    return np.ascontiguousarray(t.T).astype(np.float16)


def kernel(**inputs):
    if "nc" not in _CACHE:
        _CACHE["nc"] = _build_nc()
    nc = _CACHE["nc"]
    weights, boh = _prep_weights(inputs)

    x = np.asarray(inputs["x"])  # [8, S, S, H] fp32
    in_maps = []
    for b in range(8):
        m = dict(weights)
        m["xf"] = _prep_x(x[b])
        in_maps.append(m)

    res = run_bass_kernel_spmd(nc, in_maps, core_ids=list(range(8)))
    outs = []
    for b in range(8):
        vm = res.results[b]["out_vm"].reshape(V, S, S).astype(np.float32)
        outs.append(vm.transpose(1, 2, 0) + boh[None, None, :])
    return np.stack(outs).astype(np.float32)


# revision 26
# speedup vs baseline: 1.1267x; 1.1267x over previous
"""Trainium2 Bass kernel for nn_BondHead2 (dense_mlp), v3.

Per batch element b (8, one per NeuronCore):
    h = LN(gelu(x @ W1 + b1)); h = LN(gelu(h @ W2 + b2)); h = LN(gelu(h @ W3 + b3))
    out = h @ Wo + bo;  out = (out + out^T_{seq axes}) / 2

v3 vs v2 (all targeting the TimelineSim cost model):
  - stats batches of 8 tiles (18 batches); the vocab projection rides the
    layer-2 mean-stats matmuls for free (stat rows 0:32 + proj rows 32:112
    share one PSUM bank; matmul cost is charged per output column only).
  - gelu / square / h-mul in [128, 2048] tiles: half the Act/DVE
    instruction count of v2.
  - L0 of batch s and L1 of batch s-2 share one stats bank (64 rows) ->
    one Act drain + one fat DMA + one rd write + one repl broadcast DMA
    per step covers both layers' rstd chains.
  - DMA consolidation: the rstd broadcast is ONE DMA per step (was 8 per
    batch-layer), mobuf write one DMA per L2 batch, fat reshape one DMA.
  - unit-level software pipeline (stats lag mains by one unit, crossing
    step boundaries) so the PE never waits on Act gelu or DVE square.
  - pipeline: L0(b)@step b, L1(b)@b+2, L2(b)@b+4, drains @b+5; both
    apply_h consumers of a step's repl run exactly 2 steps later.
"""

import numpy as np

import concourse.bacc as bacc
import concourse.bass as bass
import concourse.mybir as mybir
import concourse.tile as tile
from concourse.bass_utils import run_bass_kernel_spmd

F16 = mybir.dt.float16
F32 = mybir.dt.float32
U32 = mybir.dt.uint32
AF = mybir.ActivationFunctionType
OP = mybir.AluOpType

H = 64            # hidden dim
S = 384           # seq
T = S * S         # tokens per core (147456)
V = 5             # vocab
N = 512           # tokens per matmul tile (free dim)
SB = 8            # tiles per stats batch
NB = T // (2 * SB * N)   # 18 batches
BOFF = SB * N     # batch-local offset of group B (4096)
BLK = 2 * BOFF    # tokens per batch (8192)
SBLK = 2 * BLK    # tokens per sym block (16384; block k = batches 2k, 2k+1)
EPS = 1e-5
MAGIC = 0x5F3759DF

_CACHE: dict = {}


def _build_nc():
    nc = bacc.Bacc("TRN2", target_bir_lowering=False, debug=False)

    # ---- external inputs ----
    xf = nc.dram_tensor("xf", (H, T), F16, kind="ExternalInput").ap()
    wmain = nc.dram_tensor("wmain", (128, 3 * 128), F16, kind="ExternalInput").ap()
    # per bt: cols 0:32 g-selector (means), cols 32:64 s-selector (msqs)
    wstat = nc.dram_tensor("wstat", (128, SB * 64), F16, kind="ExternalInput").ap()
    # L2 merged: per bt [128, 112]: stats-mean cols 0:32 + proj cols 32:112
    wsp = nc.dram_tensor("wsp", (128, SB * 112), F16, kind="ExternalInput").ap()
    biases = nc.dram_tensor("biases", (128, 3), F32, kind="ExternalInput").ap()
    id128 = nc.dram_tensor("id128", (128, 128), F16, kind="ExternalInput").ap()

    # ---- internal DRAM ----
    mobuf = nc.dram_tensor("mobuf", (V * T,), F16)          # [v, tok] linear
    r3buf = nc.dram_tensor("r3buf", (T,), F16)              # [tok] linear
    rdbuf = nc.dram_tensor("rdbuf", (NB + 2, 2 * BLK), F16)  # rstd per step

    out_vm = nc.dram_tensor("out_vm", (V * T,), F16, kind="ExternalOutput").ap()

    with tile.TileContext(nc) as tc:
        with tc.tile_pool(name="wpool", bufs=1) as wpool:
            from contextlib import ExitStack
            mstack = ExitStack()
            xpool = mstack.enter_context(tc.tile_pool(name="xpool", bufs=3))
            gpool = mstack.enter_context(tc.tile_pool(name="gpool", bufs=18))
            spool = mstack.enter_context(tc.tile_pool(name="spool", bufs=4))
            fpool = mstack.enter_context(tc.tile_pool(name="fpool", bufs=2))
            rpool = mstack.enter_context(tc.tile_pool(name="rpool", bufs=3))
            mps = mstack.enter_context(
                tc.tile_pool(name="mps", bufs=1, space="PSUM"))
            stps = mstack.enter_context(
                tc.tile_pool(name="stps", bufs=2, space="PSUM"))
            st2ps = mstack.enter_context(
                tc.tile_pool(name="st2ps", bufs=1, space="PSUM"))
            syps = mstack.enter_context(
                tc.tile_pool(name="syps", bufs=1, space="PSUM"))
            sy = mstack.enter_context(tc.tile_pool(name="sypool", bufs=3))

            # resident weights (wmain first: the first mains need only wm
            # and the first x chunk, so keep the DMA queue short up front)
            wm = wpool.tile([128, 3 * 128], F16)
            nc.sync.dma_start(out=wm[:], in_=wmain)
            bcol = wpool.tile([128, 3], F32)
            nc.sync.dma_start(out=bcol[:], in_=biases)
            ws = wpool.tile([128, SB * 64], F16)
            wp2 = wpool.tile([128, SB * 112], F16)
            magic = wpool.tile([128, 1], U32)
            nc.vector.memset(magic[:], MAGIC)
            oneu = wpool.tile([128, 1], U32)
            nc.vector.memset(oneu[:], 1)
            chalf = wpool.tile([128, 1], F32)
            nc.vector.memset(chalf[:], -0.5)
            c15 = wpool.tile([128, 1], F32)
            nc.vector.memset(c15[:], 1.5)
            ceps = wpool.tile([128, 1], F32)
            nc.vector.memset(ceps[:], EPS)
            idt = wpool.tile([128, 128], F16)

            gstate = {}    # (b, layer) -> [g_h0, g_h1] tiles [128, 2048]
            xstate = {}    # b -> ("split"|"whole", tiles)
            bankst = {}    # step -> paired L0/L1 stats bank [64, N]
            bank2 = {}     # b -> L2 stats+proj bank [112, N]
            replst = {}    # step -> repl tile [128, 2*BOFF]
            sunits = {}    # (b, layer, half) -> s tile
            pend = []      # stats units pending (lag 1 behind mains)

            def rsqrt_to(v_f32, out_ap, sh, tg):
                """out <- rsqrt(v) via quake seed + 1 Newton step.

                Seed bit ops on DVE (hw GPSIMD lacks 32-bit shift); the
                Newton polynomial runs on the otherwise idle Pool engine.
                """
                y32 = fpool.tile(list(sh), F32, tag="nry" + tg)
                yi = y32[:].bitcast(U32)
                vi = v_f32.bitcast(U32)
                nc.vector.tensor_tensor(
                    yi, vi, oneu[:].to_broadcast(sh), OP.logical_shift_right)
                nc.vector.tensor_tensor(
                    yi, magic[:].to_broadcast(sh), yi, OP.subtract)
                t = fpool.tile(list(sh), F32, tag="nrt" + tg)
                nc.gpsimd.tensor_mul(t[:], y32[:], y32[:])
                nc.gpsimd.tensor_mul(t[:], t[:], v_f32)
                nc.gpsimd.tensor_mul(t[:], t[:], chalf[:].to_broadcast(sh))
                nc.gpsimd.tensor_tensor(
                    t[:], t[:], c15[:].to_broadcast(sh), OP.add)
                nc.gpsimd.tensor_mul(out_ap, t[:], y32[:])

            def prefetch_x(b, split=False):
                if split:
                    xch = []
                    for c in range(4):
                        xt = xpool.tile([128, 1024], F16, tag="xs", bufs=4)
                        src = bass.AP(
                            tensor=xf.tensor,
                            offset=b * BLK + c * 1024,
                            ap=[[BOFF, 2], [T, 64], [1, 1024]],
                        )
                        nc.sync.dma_start(out=xt[:], in_=src)
                        xch.append(xt)
                    xstate[b] = ("split", xch)
                else:
                    xt = xpool.tile([128, BOFF], F16, tag="x")
                    src = bass.AP(
                        tensor=xf.tensor,
                        offset=b * BLK,
                        ap=[[BOFF, 2], [T, 64], [1, BOFF]],
                    )
                    nc.sync.dma_start(out=xt[:], in_=src)
                    xstate[b] = ("whole", xt)

            def x_slice(b, bt):
                kind, xv = xstate[b]
                if kind == "split":
                    return xv[bt // 2][:, (bt % 2) * N:(bt % 2 + 1) * N]
                return xv[:, bt * N:(bt + 1) * N]

            # ---------------- unit machinery ----------------

            def mains(b, layer, half):
                mpair = mps.tile([128, 4, N], F32, tag="m")
                for k in range(4):
                    bt = 4 * half + k
                    if layer == 0:
                        rhs = x_slice(b, bt)
                    else:
                        gh = gstate[(b, layer - 1)][bt // 4]
                        rhs = gh[:, (bt % 4) * N:(bt % 4 + 1) * N]
                    nc.tensor.matmul(
                        mpair[:, k, :],
                        wm[:, 128 * layer:128 * (layer + 1)],
                        rhs, start=True, stop=True,
                    )
                if layer > 0 and half == 1:
                    gstate.pop((b, layer - 1))
                if layer == 0 and half == 1:
                    xstate.pop(b, None)
                g = gpool.tile([128, 4 * N], F16, tag="g")
                # two gelus: banks 0-1 free as soon as the first finishes,
                # so the next unit's mains are never gated on the full tile
                nc.scalar.activation(
                    g[:, 0:2 * N],
                    mpair[:, 0:2, :].rearrange("p a n -> p (a n)"),
                    AF.Gelu, bias=bcol[:, layer:layer + 1], scale=1.0,
                )
                nc.scalar.activation(
                    g[:, 2 * N:4 * N],
                    mpair[:, 2:4, :].rearrange("p a n -> p (a n)"),
                    AF.Gelu, bias=bcol[:, layer:layer + 1], scale=1.0,
                )
                s = spool.tile([128, 4 * N], F16, tag="s", bufs=4)
                nc.vector.tensor_mul(s[:], g[:], g[:])
                gstate.setdefault((b, layer), [None, None])[half] = g
                sunits[(b, layer, half)] = s

            def stats(b, layer, half):
                """8 matmuls: mean(+proj for L2) on g, msq on s."""
                g = gstate[(b, layer)][half]
                s = sunits.pop((b, layer, half))
                if layer == 2:
                    bank = bank2[b]
                    for k in range(4):
                        bt = 4 * half + k
                        nc.tensor.matmul(
                            bank[0:112], wp2[:, 112 * bt:112 * bt + 112],
                            g[:, k * N:(k + 1) * N],
                            start=(half == 0 and k == 0), stop=False,
                            skip_group_check=True,
                        )
                        nc.tensor.matmul(
                            bank[0:32], ws[:, 64 * bt + 32:64 * bt + 64],
                            s[:, k * N:(k + 1) * N],
                            start=False, stop=(half == 1 and k == 3),
                            skip_group_check=True,
                        )
                else:
                    # L0 of batch b shares bank[64, N] with L1 of batch b-2
                    # (same step): L0 rows 0:32, L1 rows 32:64.  Each 32-row
                    # region is its own accumulation group (start=True on
                    # its first matmul resets only that region).
                    step_key = b + 2 * layer
                    bank = bankst[step_key]
                    for k in range(4):
                        bt = 4 * half + k
                        nc.tensor.matmul(
                            bank[32 * layer:32 * layer + 32],
                            ws[:, 64 * bt:64 * bt + 32],
                            g[:, k * N:(k + 1) * N],
                            start=(half == 0 and k == 0), stop=False,
                            skip_group_check=True,
                        )
                        nc.tensor.matmul(
                            bank[32 * layer:32 * layer + 32],
                            ws[:, 64 * bt + 32:64 * bt + 64],
                            s[:, k * N:(k + 1) * N],
                            start=False, stop=(half == 1 and k == 3),
                            skip_group_check=True,
                        )

            # ---------------- stats postprocessing ----------------
            # Bank row layout per 32-row region: 0:8 grpA-mean(bt), 8:16
            # grpB-mean, 16:24 grpA-msq, 24:32 grpB-msq; bank col = c (512).
            # fat layout: partition p = 64l + 32g + 4bt + c//128, giving a
            # FLAT rf index = group-local token, so the rd write is linear.

            def drain_pair(step):
                bank = bankst.pop(step)
                rowb = fpool.tile([64, N], F16, tag="rowb")
                nc.scalar.copy(rowb[:], bank[:])
                # mean rows 32l + 8g + bt (msq rows between the l blocks)
                # -> meanfat flat index = l*BLK + g*BOFF + bt*N + c
                meanfat = fpool.tile([128, 128], F16, tag="meanfat")
                msqfat = fpool.tile([128, 128], F16, tag="msqfat")
                for l in range(2):
                    nc.sync.dma_start(
                        out=meanfat[64 * l:64 * (l + 1), :],
                        in_=rowb[32 * l:32 * l + 16, :]
                        .rearrange("p (q c) -> p q c", q=4),
                    )
                    nc.sync.dma_start(
                        out=msqfat[64 * l:64 * (l + 1), :],
                        in_=rowb[32 * l + 16:32 * l + 32, :]
                        .rearrange("p (q c) -> p q c", q=4),
                    )
                # var = msq - mean^2 + eps  (fp32, on Pool)
                sqf = fpool.tile([128, 128], F32, tag="sqf")
                nc.gpsimd.tensor_mul(sqf[:], meanfat[:], meanfat[:])
                varf = fpool.tile([128, 128], F32, tag="varf")
                nc.gpsimd.tensor_tensor(
                    varf[:], msqfat[:], sqf[:], OP.subtract)
                nc.gpsimd.tensor_tensor(
                    varf[:], varf[:], ceps[:].to_broadcast((128, 128)), OP.add)
                rf = fpool.tile([128, 128], F16, tag="rf")
                rsqrt_to(varf[:], rf[:], (128, 128), "p")
                # rf flat index == l*BLK + g*BOFF + bt*N + c  -> linear write
                nc.sync.dma_start(
                    out=bass.AP(tensor=rdbuf, offset=step * 2 * BLK,
                                ap=[[128, 128], [1, 128]]),
                    in_=rf[:],
                )
                repl = rpool.tile([128, 2 * BOFF], F16, tag="repl")
                for l in range(2):
                    src2 = bass.AP(
                        tensor=rdbuf, offset=step * 2 * BLK + l * BLK,
                        # dims (grp, dup64, c)
                        ap=[[BOFF, 2], [0, 64], [1, BOFF]],
                    )
                    nc.sync.dma_start(
                        out=repl[:, l * BOFF:(l + 1) * BOFF], in_=src2)
                replst[step] = repl

            def apply_h(step, b, layer):
                """h = g * rstd in place (repl cols: L0 0:BOFF, L1 BOFF:)."""
                repl = replst[step]
                off = layer * BOFF
                for half in range(2):
                    g = gstate[(b, layer)][half]
                    nc.vector.tensor_mul(
                        g[:], g[:],
                        repl[:, off + half * 4 * N: off + (half + 1) * 4 * N])

            def drain_l2(b):
                """L2 bank: stats rows 0:32 -> r3buf; proj rows 32:112 -> mobuf."""
                bank = bank2.pop(b)
                dr = fpool.tile([112, N], F16, tag="dr2", bufs=3)
                nc.scalar.copy(dr[:], bank[:])
                gstate.pop((b, 2), None)
                # proj rows 32:112 are (g, v, bt)-ordered: plain read, the
                # structure lives on the DRAM side: dims (g, v, bt, c)
                for g2 in range(2):
                    dst = bass.AP(
                        tensor=mobuf, offset=b * BLK + g2 * BOFF,
                        ap=[[T, V], [1, BOFF]],
                    )
                    nc.sync.dma_start(
                        out=dst, in_=dr[32 + 40 * g2:72 + 40 * g2, :])
                # stats: mean rows 0:16 / msq rows 16:32, both linear in
                # (g, bt, c) -> [128, 64] fats with flat index = token
                mean2 = fpool.tile([128, 64], F16, tag="mean2")
                nc.sync.dma_start(
                    out=mean2[:],
                    in_=dr[0:16, :].rearrange("p (q c) -> p q c", q=8),
                )
                msq2 = fpool.tile([128, 64], F16, tag="msq2")
                nc.sync.dma_start(
                    out=msq2[:],
                    in_=dr[16:32, :].rearrange("p (q c) -> p q c", q=8),
                )
                sqf = fpool.tile([128, 64], F32, tag="sqf2")
                nc.gpsimd.tensor_mul(sqf[:], mean2[:], mean2[:])
                varf = fpool.tile([128, 64], F32, tag="varf2")
                nc.gpsimd.tensor_tensor(
                    varf[:], msq2[:], sqf[:], OP.subtract)
                nc.gpsimd.tensor_tensor(
                    varf[:], varf[:], ceps[:].to_broadcast((128, 64)), OP.add)
                rf3 = fpool.tile([128, 64], F16, tag="rf3")
                rsqrt_to(varf[:], rf3[:], (128, 64), "3")
                # rf3 flat index == group-local token -> linear write
                nc.sync.dma_start(
                    out=bass.AP(tensor=r3buf, offset=b * BLK,
                                ap=[[64, 128], [1, 64]]),
                    in_=rf3[:],
                )

            # ---------------- symmetrization ----------------
            sym_pmap = {}

            def sym_prep(blk):
                mo = sy.tile([128, V, 128], F16, tag="mo_in", bufs=2)
                src = bass.AP(
                    tensor=mobuf, offset=blk * SBLK,
                    ap=[[128, 128], [T, V], [1, 128]],
                )
                nc.sync.dma_start(out=mo[:], in_=src)
                r = sy.tile([128, 128], F16, tag="r_in", bufs=2)
                rsrc = bass.AP(
                    tensor=r3buf, offset=blk * SBLK,
                    ap=[[128, 128], [1, 128]],
                )
                nc.sync.dma_start(out=r[:], in_=rsrc)
                p_ = sy.tile([128, V, 128], F16, tag="p", bufs=5)
                ra = r[:]
                rbc = bass.AP(tensor=ra.tensor, offset=ra.offset,
                              ap=[ra.ap[0], [0, V], ra.ap[1]])
                nc.vector.tensor_mul(p_[:], mo[:], rbc)
                sym_pmap[blk] = p_

            def sym_transposes(p_):
                pt = syps.tile([128, V, 128], F16, tag="pt")
                for v in range(V):
                    nc.tensor.transpose(pt[:, v, :], p_[:, v, :], idt[:])
                return pt

            def sym_emit(pa, pt, bi, bj):
                o = sy.tile([128, V, 128], F16, tag="o", bufs=2)
                nc.vector.tensor_add(
                    o[:].rearrange("p a n -> p (a n)"),
                    pa[:].rearrange("p a n -> p (a n)"),
                    pt[:].rearrange("p a n -> p (a n)"),
                )
                d1 = bass.AP(
                    tensor=out_vm.tensor, offset=bi * 128 * S + bj * 128,
                    ap=[[S, 128], [T, V], [1, 128]],
                )
                nc.sync.dma_start(out=d1, in_=o[:])

            def sym_pair(bi, bj):
                pa = sym_pmap.pop(3 * bi + bj)
                if bi == bj:
                    sym_emit(pa, sym_transposes(pa), bi, bj)
                else:
                    pb = sym_pmap.pop(3 * bj + bi)
                    sym_emit(pa, sym_transposes(pb), bi, bj)
                    sym_emit(pb, sym_transposes(pa), bj, bi)

            # sym block k = batches 2k, 2k+1; r3/mobuf of batch b land in
            # step b+4, so block k is ready at step 2k+5.
            sym_sched = {}
            sym_prep_sched = {}
            for bi in range(3):
                for bj in range(bi + 1):
                    gate = 2 * max(3 * bi + bj, 3 * bj + bi) + 6
                    sym_sched.setdefault(gate, []).append((bi, bj))
                    for blk in {3 * bi + bj, 3 * bj + bi}:
                        sym_prep_sched.setdefault(gate, []).append(blk)

            prefetch_x(0, split=True)
            nc.sync.dma_start(out=ws[:], in_=wstat)
            nc.sync.dma_start(out=wp2[:], in_=wsp)
            nc.sync.dma_start(out=idt[:], in_=id128)

            # pipeline: L0(b)@b, L1(b)@b+2, L2(b)@b+3; drain_l2(b)@b+4
            # (early in the step, so the freed bank never stalls this
            # step's L2 stats); stats lag mains by one unit crossing step
            # boundaries; apply_h(L0(b)) runs in step b+1's DVE tail,
            # apply_h(L1(b)) early in step b+3 (interleaved after unit 1
            # so it never heads-of-line-blocks the squares).
            def emit_step(step):
                units = []
                if step < NB:
                    units.append((step, 0, 0))
                    units.append((step, 0, 1))
                if 0 <= step - 2 < NB:
                    units.append((step - 2, 1, 0))
                    units.append((step - 2, 1, 1))
                if 0 <= step - 4 < NB:
                    units.append((step - 4, 2, 0))
                    units.append((step - 4, 2, 1))

                has_l0 = step < NB
                has_l1 = 0 <= step - 2 < NB
                if has_l0 or has_l1:
                    bk = stps.tile([64, N], F32, tag="stat", name="bk")
                    bankst[step] = bk
                    if not has_l0:
                        nc.vector.memset(bk[0:32], 0.0)
                    if not has_l1:
                        nc.vector.memset(bk[32:64], 0.0)
                if 0 <= step - 4 < NB:
                    bank2[step - 4] = st2ps.tile([112, N], F32, tag="st2",
                                                 name="bk2")

                if step + 2 < NB:
                    prefetch_x(step + 2)

                def unit(i):
                    mains(*units[i])
                    if pend:
                        stats(*pend.pop(0))
                    pend.append(units[i])

                # both h-multiplies use repl(step-2) (fully landed); they
                # are interleaved into the DVE queue just before their
                # consumer units so they never head-of-line-block squares
                idx_l1 = next((i for i, u in enumerate(units)
                               if u[1] == 1), None)
                idx_l2 = next((i for i, u in enumerate(units)
                               if u[1] == 2), None)

                if units:
                    if idx_l1 == 0 and 0 <= step - 2 < NB:
                        apply_h(step - 2, step - 2, 0)
                    if idx_l2 == 0 and 0 <= step - 4 < NB:
                        apply_h(step - 2, step - 4, 1)
                    unit(0)
                else:
                    while pend:
                        stats(*pend.pop(0))
                # L2h1(step-5)'s stats were just flushed -> its bank drains
                # now, freeing st2ps well before this step's L2 stats
                if 0 <= step - 5 < NB:
                    drain_l2(step - 5)
                for i in range(1, len(units)):
                    if i == idx_l1 and 0 <= step - 2 < NB:
                        apply_h(step - 2, step - 2, 0)
                    if i == idx_l2 and 0 <= step - 4 < NB:
                        apply_h(step - 2, step - 4, 1)
                    unit(i)
                # flush any pending unit whose bank drains this step (only
                # happens in warmup/tail steps; steady state keeps the lag)
                while pend and pend[0][0] + (0, 2, 5)[pend[0][1]] <= step:
                    stats(*pend.pop(0))

                if step in bankst:
                    drain_pair(step)

                for blk in sym_prep_sched.get(step, []):
                    sym_prep(blk)
                for (bi, bj) in sym_sched.get(step, []):
                    sym_pair(bi, bj)
                replst.pop(step - 3, None)

            prefetch_x(1)
            for step in range(NB + 5):
                emit_step(step)
            mstack.close()

    nc.compile()
    return nc


def _prep_weights(inputs):
    W1 = np.asarray(inputs["W1"], np.float64)
    W2 = np.asarray(inputs["W2"], np.float64)
    W3 = np.asarray(inputs["W3"], np.float64)
    Wo = np.asarray(inputs["Wo"], np.float64)
    b1 = np.asarray(inputs["b1"], np.float64)
    b2 = np.asarray(inputs["b2"], np.float64)
    b3 = np.asarray(inputs["b3"], np.float64)
    bo = np.asarray(inputs["bo"], np.float64)
    ln_g = np.asarray(inputs["ln_g"], np.float64)
    ln_b = np.asarray(inputs["ln_b"], np.float64)

    C = np.eye(H) - np.ones((H, H)) / H
    F = C @ np.diag(ln_g)
    Ws = [W1, F @ W2, F @ W3]
    bs = [b1, b2 + W2.T @ ln_b, b3 + W3.T @ ln_b]
    Woh = 0.5 * (F @ Wo)
    boh = (bo + Wo.T @ ln_b).astype(np.float32)

    wmain = np.zeros((128, 3 * 128), np.float16)
    for l, W in enumerate(Ws):
        wmain[0:64, 128 * l:128 * l + 64] = W.astype(np.float16)
        wmain[64:128, 128 * l + 64:128 * l + 128] = W.astype(np.float16)

    # per bt: g-selector cols 0:32 (rows 0:8 A-mean(bt), 8:16 B-mean),
    # s-selector cols 32:64 (rows 16:24 A-msq, 24:32 B-msq)
    wstat = np.zeros((128, SB * 64), np.float16)
    for bt in range(SB):
        wstat[0:64, 64 * bt + bt] = np.float16(1 / 64)
        wstat[64:128, 64 * bt + 8 + bt] = np.float16(1 / 64)
        wstat[0:64, 64 * bt + 32 + 16 + bt] = np.float16(1 / 64)
        wstat[64:128, 64 * bt + 32 + 24 + bt] = np.float16(1 / 64)

    # L2 merged: per bt [128, 112]: mean cols 0:32 + proj rows
    # 32 + 40g + 8v + bt
    wsp = np.zeros((128, SB * 112), np.float16)
    w16 = Woh.astype(np.float16)
    for bt in range(SB):
        wsp[0:64, 112 * bt + bt] = np.float16(1 / 64)
        wsp[64:128, 112 * bt + 8 + bt] = np.float16(1 / 64)
        for v in range(V):
            wsp[0:64, 112 * bt + 32 + 8 * v + bt] = w16[:, v]
            wsp[64:128, 112 * bt + 32 + 40 + 8 * v + bt] = w16[:, v]

    biases = np.zeros((128, 3), np.float32)
    for l, bb in enumerate(bs):
        biases[0:64, l] = bb.astype(np.float32)
        biases[64:128, l] = bb.astype(np.float32)
    id128 = np.eye(128, dtype=np.float16)
    return dict(wmain=wmain, wstat=wstat, wsp=wsp, biases=biases,
                id128=id128), boh


def _prep_x(xb):
    """[S, S, H] fp32 -> [H, T] fp16 in block-major token order."""
    t = xb.reshape(3, 128, 3, 128, H).transpose(0, 2, 1, 3, 4).reshape(T, H)
    return np.ascontiguousarray(t.T).astype(np.float16)


def kernel(**inputs):
    if "nc" not in _CACHE:
        _CACHE["nc"] = _build_nc()
    nc = _CACHE["nc"]
    weights, boh = _prep_weights(inputs)

    x = np.asarray(inputs["x"])  # [8, S, S, H] fp32
    in_maps = []
    for b in range(8):
        m = dict(weights)
        m["xf"] = _prep_x(x[b])
        in_maps.append(m)

    res = run_bass_kernel_spmd(nc, in_maps, core_ids=list(range(8)))
    outs = []
    for b in range(8):
        vm = res.results[b]["out_vm"].reshape(V, S, S).astype(np.float32)
        outs.append(vm.transpose(1, 2, 0) + boh[None, None, :])
    return np.stack(outs).astype(np.float32)


# revision 30
# speedup vs baseline: 1.2160x; 1.0793x over previous
"""Trainium2 Bass kernel for nn_BondHead2 (dense_mlp), v3.

Per batch element b (8, one per NeuronCore):
    h = LN(gelu(x @ W1 + b1)); h = LN(gelu(h @ W2 + b2)); h = LN(gelu(h @ W3 + b3))
    out = h @ Wo + bo;  out = (out + out^T_{seq axes}) / 2

v3 vs v2 (all targeting the TimelineSim cost model):
  - stats batches of 8 tiles (18 batches); the vocab projection rides the
    layer-2 mean-stats matmuls for free (stat rows 0:32 + proj rows 32:112
    share one PSUM bank; matmul cost is charged per output column only).
  - gelu / square / h-mul in [128, 2048] tiles: half the Act/DVE
    instruction count of v2.
  - L0 of batch s and L1 of batch s-2 share one stats bank (64 rows) ->
    one Act drain + one fat DMA + one rd write + one repl broadcast DMA
    per step covers both layers' rstd chains.
  - DMA consolidation: the rstd broadcast is ONE DMA per step (was 8 per
    batch-layer), mobuf write one DMA per L2 batch, fat reshape one DMA.
  - unit-level software pipeline (stats lag mains by one unit, crossing
    step boundaries) so the PE never waits on Act gelu or DVE square.
  - pipeline: L0(b)@step b, L1(b)@b+2, L2(b)@b+4, drains @b+5; both
    apply_h consumers of a step's repl run exactly 2 steps later.
"""

import numpy as np

import concourse.bacc as bacc
import concourse.bass as bass
import concourse.mybir as mybir
import concourse.tile as tile
from concourse.bass_utils import run_bass_kernel_spmd

F16 = mybir.dt.float16
F32 = mybir.dt.float32
U32 = mybir.dt.uint32
AF = mybir.ActivationFunctionType
OP = mybir.AluOpType

H = 64            # hidden dim
S = 384           # seq
T = S * S         # tokens per core (147456)
V = 5             # vocab
N = 512           # tokens per matmul tile (free dim)
SB = 8            # tiles per stats batch
NB = T // (2 * SB * N)   # 18 batches
BOFF = SB * N     # batch-local offset of group B (4096)
BLK = 2 * BOFF    # tokens per batch (8192)
SBLK = 2 * BLK    # tokens per sym block (16384; block k = batches 2k, 2k+1)
EPS = 1e-5
MAGIC = 0x5F3759DF

_CACHE: dict = {}


def _build_nc():
    nc = bacc.Bacc("TRN2", target_bir_lowering=False, debug=False)

    # ---- external inputs ----
    xf = nc.dram_tensor("xf", (H, T), F16, kind="ExternalInput").ap()
    wmain = nc.dram_tensor("wmain", (128, 3 * 128), F16, kind="ExternalInput").ap()
    # per bt: cols 0:32 g-selector (means), cols 32:64 s-selector (msqs)
    wstat = nc.dram_tensor("wstat", (128, SB * 64), F16, kind="ExternalInput").ap()
    # L2 merged: per bt [128, 112]: stats-mean cols 0:32 + proj cols 32:112
    wsp = nc.dram_tensor("wsp", (128, SB * 112), F16, kind="ExternalInput").ap()
    biases = nc.dram_tensor("biases", (128, 3), F32, kind="ExternalInput").ap()
    id128 = nc.dram_tensor("id128", (128, 128), F16, kind="ExternalInput").ap()

    # ---- internal DRAM ----
    mobuf = nc.dram_tensor("mobuf", (V * T,), F16)          # [v, tok] linear
    r3buf = nc.dram_tensor("r3buf", (T,), F16)              # [tok] linear
    rdbuf = nc.dram_tensor("rdbuf", (NB + 2, 2 * BLK), F16)  # rstd per step

    out_vm = nc.dram_tensor("out_vm", (V * T,), F16, kind="ExternalOutput").ap()

    with tile.TileContext(nc) as tc:
        with tc.tile_pool(name="wpool", bufs=1) as wpool:
            from contextlib import ExitStack
            mstack = ExitStack()
            xpool = mstack.enter_context(tc.tile_pool(name="xpool", bufs=3))
            gpool = mstack.enter_context(tc.tile_pool(name="gpool", bufs=18))
            spool = mstack.enter_context(tc.tile_pool(name="spool", bufs=4))
            fpool = mstack.enter_context(tc.tile_pool(name="fpool", bufs=2))
            rpool = mstack.enter_context(tc.tile_pool(name="rpool", bufs=3))
            mpsa = mstack.enter_context(
                tc.tile_pool(name="mpsa", bufs=1, space="PSUM"))
            mpsb = mstack.enter_context(
                tc.tile_pool(name="mpsb", bufs=1, space="PSUM"))
            stps = mstack.enter_context(
                tc.tile_pool(name="stps", bufs=2, space="PSUM"))
            st2ps = mstack.enter_context(
                tc.tile_pool(name="st2ps", bufs=1, space="PSUM"))
            syps = mstack.enter_context(
                tc.tile_pool(name="syps", bufs=1, space="PSUM"))
            sy = mstack.enter_context(tc.tile_pool(name="sypool", bufs=3))

            # resident weights (wmain first: the first mains need only wm
            # and the first x chunk, so keep the DMA queue short up front)
            wm = wpool.tile([128, 3 * 128], F16)
            nc.sync.dma_start(out=wm[:], in_=wmain)
            bcol = wpool.tile([128, 3], F32)
            nc.sync.dma_start(out=bcol[:], in_=biases)
            ws = wpool.tile([128, SB * 64], F16)
            wp2 = wpool.tile([128, SB * 112], F16)
            magic = wpool.tile([128, 1], U32)
            nc.vector.memset(magic[:], MAGIC)
            oneu = wpool.tile([128, 1], U32)
            nc.vector.memset(oneu[:], 1)
            chalf = wpool.tile([128, 1], F32)
            nc.vector.memset(chalf[:], -0.5)
            c15 = wpool.tile([128, 1], F32)
            nc.vector.memset(c15[:], 1.5)
            ceps = wpool.tile([128, 1], F32)
            nc.vector.memset(ceps[:], EPS)
            idt = wpool.tile([128, 128], F16)

            gstate = {}    # (b, layer) -> [g_h0, g_h1] tiles [128, 2048]
            xstate = {}    # b -> ("split"|"whole", tiles)
            bankst = {}    # step -> paired L0/L1 stats bank [64, N]
            bank2 = {}     # b -> L2 stats+proj bank [112, N]
            replst = {}    # step -> repl tile [128, 2*BOFF]
            sunits = {}    # (b, layer, half) -> s tile
            pend = []      # stats units pending (lag 1 behind mains)

            def rsqrt_to(v_f32, out_ap, sh, tg):
                """out <- rsqrt(v) via quake seed + 1 Newton step.

                Seed bit ops on DVE (hw GPSIMD lacks 32-bit shift); the
                Newton polynomial runs on the otherwise idle Pool engine.
                """
                y32 = fpool.tile(list(sh), F32, tag="nry" + tg)
                yi = y32[:].bitcast(U32)
                vi = v_f32.bitcast(U32)
                nc.vector.tensor_tensor(
                    yi, vi, oneu[:].to_broadcast(sh), OP.logical_shift_right)
                nc.vector.tensor_tensor(
                    yi, magic[:].to_broadcast(sh), yi, OP.subtract)
                t = fpool.tile(list(sh), F32, tag="nrt" + tg)
                nc.gpsimd.tensor_mul(t[:], y32[:], y32[:])
                nc.gpsimd.tensor_mul(t[:], t[:], v_f32)
                nc.gpsimd.tensor_mul(t[:], t[:], chalf[:].to_broadcast(sh))
                nc.gpsimd.tensor_tensor(
                    t[:], t[:], c15[:].to_broadcast(sh), OP.add)
                nc.gpsimd.tensor_mul(out_ap, t[:], y32[:])

            def prefetch_x(b, split=False):
                if split:
                    xch = []
                    for c in range(4):
                        xt = xpool.tile([128, 1024], F16, tag="xs", bufs=4)
                        src = bass.AP(
                            tensor=xf.tensor,
                            offset=b * BLK + c * 1024,
                            ap=[[BOFF, 2], [T, 64], [1, 1024]],
                        )
                        nc.sync.dma_start(out=xt[:], in_=src)
                        xch.append(xt)
                    xstate[b] = ("split", xch)
                else:
                    xt = xpool.tile([128, BOFF], F16, tag="x")
                    src = bass.AP(
                        tensor=xf.tensor,
                        offset=b * BLK,
                        ap=[[BOFF, 2], [T, 64], [1, BOFF]],
                    )
                    nc.sync.dma_start(out=xt[:], in_=src)
                    xstate[b] = ("whole", xt)

            def x_slice(b, bt):
                kind, xv = xstate[b]
                if kind == "split":
                    return xv[bt // 2][:, (bt % 2) * N:(bt % 2 + 1) * N]
                return xv[:, bt * N:(bt + 1) * N]

            # ---------------- unit machinery ----------------

            def mains(b, layer, half):
                # two alternating 2-bank PSUM tiles: pool rotation is
                # tile-granular, so each gelu only gates the SAME pool's
                # mains two half-units later (plenty of slack)
                g = gpool.tile([128, 4 * N], F16, tag="g")
                for p, pool in enumerate((mpsa, mpsb)):
                    mpair = pool.tile([128, 2, N], F32, tag="m", name="mp")
                    for k in range(2):
                        bt = 4 * half + 2 * p + k
                        if layer == 0:
                            rhs = x_slice(b, bt)
                        else:
                            gh = gstate[(b, layer - 1)][bt // 4]
                            rhs = gh[:, (bt % 4) * N:(bt % 4 + 1) * N]
                        nc.tensor.matmul(
                            mpair[:, k, :],
                            wm[:, 128 * layer:128 * (layer + 1)],
                            rhs, start=True, stop=True,
                        )
                    nc.scalar.activation(
                        g[:, 2 * N * p:2 * N * (p + 1)],
                        mpair[:].rearrange("p a n -> p (a n)"),
                        AF.Gelu, bias=bcol[:, layer:layer + 1], scale=1.0,
                    )
                if layer > 0 and half == 1:
                    gstate.pop((b, layer - 1))
                if layer == 0 and half == 1:
                    xstate.pop(b, None)
                s = spool.tile([128, 4 * N], F16, tag="s", bufs=4)
                nc.vector.tensor_mul(s[:], g[:], g[:])
                gstate.setdefault((b, layer), [None, None])[half] = g
                sunits[(b, layer, half)] = s

            def stats(b, layer, half):
                """8 matmuls: mean(+proj for L2) on g, msq on s."""
                g = gstate[(b, layer)][half]
                s = sunits.pop((b, layer, half))
                if layer == 2:
                    bank = bank2[b]
                    for k in range(4):
                        bt = 4 * half + k
                        nc.tensor.matmul(
                            bank[0:112], wp2[:, 112 * bt:112 * bt + 112],
                            g[:, k * N:(k + 1) * N],
                            start=(half == 0 and k == 0), stop=False,
                            skip_group_check=True,
                        )
                        nc.tensor.matmul(
                            bank[0:32], ws[:, 64 * bt + 32:64 * bt + 64],
                            s[:, k * N:(k + 1) * N],
                            start=False, stop=(half == 1 and k == 3),
                            skip_group_check=True,
                        )
                else:
                    # L0 of batch b shares bank[64, N] with L1 of batch b-2
                    # (same step): L0 rows 0:32, L1 rows 32:64.  Each 32-row
                    # region is its own accumulation group (start=True on
                    # its first matmul resets only that region).
                    step_key = b + 2 * layer
                    bank = bankst[step_key]
                    for k in range(4):
                        bt = 4 * half + k
                        nc.tensor.matmul(
                            bank[32 * layer:32 * layer + 32],
                            ws[:, 64 * bt:64 * bt + 32],
                            g[:, k * N:(k + 1) * N],
                            start=(half == 0 and k == 0), stop=False,
                            skip_group_check=True,
                        )
                        nc.tensor.matmul(
                            bank[32 * layer:32 * layer + 32],
                            ws[:, 64 * bt + 32:64 * bt + 64],
                            s[:, k * N:(k + 1) * N],
                            start=False, stop=(half == 1 and k == 3),
                            skip_group_check=True,
                        )

            # ---------------- stats postprocessing ----------------
            # Bank row layout per 32-row region: 0:8 grpA-mean(bt), 8:16
            # grpB-mean, 16:24 grpA-msq, 24:32 grpB-msq; bank col = c (512).
            # fat layout: partition p = 64l + 32g + 4bt + c//128, giving a
            # FLAT rf index = group-local token, so the rd write is linear.

            def drain_pair(step):
                bank = bankst.pop(step)
                rowb = fpool.tile([64, N], F16, tag="rowb")
                nc.scalar.copy(rowb[:], bank[:])
                # mean rows 32l + 8g + bt (msq rows between the l blocks)
                # -> meanfat flat index = l*BLK + g*BOFF + bt*N + c
                meanfat = fpool.tile([128, 128], F16, tag="meanfat")
                msqfat = fpool.tile([128, 128], F16, tag="msqfat")
                for l in range(2):
                    nc.sync.dma_start(
                        out=meanfat[64 * l:64 * (l + 1), :],
                        in_=rowb[32 * l:32 * l + 16, :]
                        .rearrange("p (q c) -> p q c", q=4),
                    )
                    nc.sync.dma_start(
                        out=msqfat[64 * l:64 * (l + 1), :],
                        in_=rowb[32 * l + 16:32 * l + 32, :]
                        .rearrange("p (q c) -> p q c", q=4),
                    )
                # var = msq - mean^2 + eps  (fp32, on Pool)
                sqf = fpool.tile([128, 128], F32, tag="sqf")
                nc.gpsimd.tensor_mul(sqf[:], meanfat[:], meanfat[:])
                varf = fpool.tile([128, 128], F32, tag="varf")
                nc.gpsimd.tensor_tensor(
                    varf[:], msqfat[:], sqf[:], OP.subtract)
                nc.gpsimd.tensor_tensor(
                    varf[:], varf[:], ceps[:].to_broadcast((128, 128)), OP.add)
                rf = fpool.tile([128, 128], F16, tag="rf")
                rsqrt_to(varf[:], rf[:], (128, 128), "p")
                # rf flat index == l*BLK + g*BOFF + bt*N + c  -> linear write
                nc.sync.dma_start(
                    out=bass.AP(tensor=rdbuf, offset=step * 2 * BLK,
                                ap=[[128, 128], [1, 128]]),
                    in_=rf[:],
                )
                repl = rpool.tile([128, 2 * BOFF], F16, tag="repl")
                for l in range(2):
                    src2 = bass.AP(
                        tensor=rdbuf, offset=step * 2 * BLK + l * BLK,
                        # dims (grp, dup64, c)
                        ap=[[BOFF, 2], [0, 64], [1, BOFF]],
                    )
                    nc.sync.dma_start(
                        out=repl[:, l * BOFF:(l + 1) * BOFF], in_=src2)
                replst[step] = repl

            def apply_h(step, b, layer):
                """h = g * rstd in place (repl cols: L0 0:BOFF, L1 BOFF:)."""
                repl = replst[step]
                off = layer * BOFF
                for half in range(2):
                    g = gstate[(b, layer)][half]
                    nc.vector.tensor_mul(
                        g[:], g[:],
                        repl[:, off + half * 4 * N: off + (half + 1) * 4 * N])

            def drain_l2(b):
                """L2 bank: stats rows 0:32 -> r3buf; proj rows 32:112 -> mobuf."""
                bank = bank2.pop(b)
                dr = fpool.tile([112, N], F16, tag="dr2", bufs=3)
                nc.scalar.copy(dr[:], bank[:])
                gstate.pop((b, 2), None)
                # proj rows 32:112 are (g, v, bt)-ordered: plain read, the
                # structure lives on the DRAM side: dims (g, v, bt, c)
                for g2 in range(2):
                    dst = bass.AP(
                        tensor=mobuf, offset=b * BLK + g2 * BOFF,
                        ap=[[T, V], [1, BOFF]],
                    )
                    nc.sync.dma_start(
                        out=dst, in_=dr[32 + 40 * g2:72 + 40 * g2, :])
                # stats: mean rows 0:16 / msq rows 16:32, both linear in
                # (g, bt, c) -> [128, 64] fats with flat index = token
                mean2 = fpool.tile([128, 64], F16, tag="mean2")
                nc.sync.dma_start(
                    out=mean2[:],
                    in_=dr[0:16, :].rearrange("p (q c) -> p q c", q=8),
                )
                msq2 = fpool.tile([128, 64], F16, tag="msq2")
                nc.sync.dma_start(
                    out=msq2[:],
                    in_=dr[16:32, :].rearrange("p (q c) -> p q c", q=8),
                )
                sqf = fpool.tile([128, 64], F32, tag="sqf2")
                nc.gpsimd.tensor_mul(sqf[:], mean2[:], mean2[:])
                varf = fpool.tile([128, 64], F32, tag="varf2")
                nc.gpsimd.tensor_tensor(
                    varf[:], msq2[:], sqf[:], OP.subtract)
                nc.gpsimd.tensor_tensor(
                    varf[:], varf[:], ceps[:].to_broadcast((128, 64)), OP.add)
                rf3 = fpool.tile([128, 64], F16, tag="rf3")
                rsqrt_to(varf[:], rf3[:], (128, 64), "3")
                # rf3 flat index == group-local token -> linear write
                nc.sync.dma_start(
                    out=bass.AP(tensor=r3buf, offset=b * BLK,
                                ap=[[64, 128], [1, 64]]),
                    in_=rf3[:],
                )

            # ---------------- symmetrization ----------------
            sym_pmap = {}

            def sym_prep(blk):
                mo = sy.tile([128, V, 128], F16, tag="mo_in", bufs=2)
                src = bass.AP(
                    tensor=mobuf, offset=blk * SBLK,
                    ap=[[128, 128], [T, V], [1, 128]],
                )
                nc.sync.dma_start(out=mo[:], in_=src)
                r = sy.tile([128, 128], F16, tag="r_in", bufs=2)
                rsrc = bass.AP(
                    tensor=r3buf, offset=blk * SBLK,
                    ap=[[128, 128], [1, 128]],
                )
                nc.sync.dma_start(out=r[:], in_=rsrc)
                p_ = sy.tile([128, V, 128], F16, tag="p", bufs=5)
                ra = r[:]
                rbc = bass.AP(tensor=ra.tensor, offset=ra.offset,
                              ap=[ra.ap[0], [0, V], ra.ap[1]])
                nc.vector.tensor_mul(p_[:], mo[:], rbc)
                sym_pmap[blk] = p_

            def sym_transposes(p_):
                pt = syps.tile([128, V, 128], F16, tag="pt")
                for v in range(V):
                    nc.tensor.transpose(pt[:, v, :], p_[:, v, :], idt[:])
                return pt

            def sym_emit(pa, pt, bi, bj):
                o = sy.tile([128, V, 128], F16, tag="o", bufs=2)
                nc.vector.tensor_add(
                    o[:].rearrange("p a n -> p (a n)"),
                    pa[:].rearrange("p a n -> p (a n)"),
                    pt[:].rearrange("p a n -> p (a n)"),
                )
                d1 = bass.AP(
                    tensor=out_vm.tensor, offset=bi * 128 * S + bj * 128,
                    ap=[[S, 128], [T, V], [1, 128]],
                )
                nc.sync.dma_start(out=d1, in_=o[:])

            def sym_pair(bi, bj):
                pa = sym_pmap.pop(3 * bi + bj)
                if bi == bj:
                    sym_emit(pa, sym_transposes(pa), bi, bj)
                else:
                    pb = sym_pmap.pop(3 * bj + bi)
                    sym_emit(pa, sym_transposes(pb), bi, bj)
                    sym_emit(pb, sym_transposes(pa), bj, bi)

            # sym block k = batches 2k, 2k+1; r3/mobuf of batch b land in
            # step b+4, so block k is ready at step 2k+5.
            sym_sched = {}
            sym_prep_sched = {}
            for bi in range(3):
                for bj in range(bi + 1):
                    gate = 2 * max(3 * bi + bj, 3 * bj + bi) + 6
                    sym_sched.setdefault(gate, []).append((bi, bj))
                    for blk in {3 * bi + bj, 3 * bj + bi}:
                        sym_prep_sched.setdefault(gate, []).append(blk)

            prefetch_x(0, split=True)
            nc.sync.dma_start(out=ws[:], in_=wstat)
            nc.sync.dma_start(out=wp2[:], in_=wsp)
            nc.sync.dma_start(out=idt[:], in_=id128)

            # pipeline: L0(b)@b, L1(b)@b+2, L2(b)@b+3; drain_l2(b)@b+4
            # (early in the step, so the freed bank never stalls this
            # step's L2 stats); stats lag mains by one unit crossing step
            # boundaries; apply_h(L0(b)) runs in step b+1's DVE tail,
            # apply_h(L1(b)) early in step b+3 (interleaved after unit 1
            # so it never heads-of-line-blocks the squares).
            def emit_step(step):
                units = []
                if step < NB:
                    units.append((step, 0, 0))
                    units.append((step, 0, 1))
                if 0 <= step - 2 < NB:
                    units.append((step - 2, 1, 0))
                    units.append((step - 2, 1, 1))
                if 0 <= step - 4 < NB:
                    units.append((step - 4, 2, 0))
                    units.append((step - 4, 2, 1))

                has_l0 = step < NB
                has_l1 = 0 <= step - 2 < NB
                if has_l0 or has_l1:
                    bk = stps.tile([64, N], F32, tag="stat", name="bk")
                    bankst[step] = bk
                    if not has_l0:
                        nc.vector.memset(bk[0:32], 0.0)
                    if not has_l1:
                        nc.vector.memset(bk[32:64], 0.0)
                if 0 <= step - 4 < NB:
                    bank2[step - 4] = st2ps.tile([112, N], F32, tag="st2",
                                                 name="bk2")

                if step + 2 < NB:
                    prefetch_x(step + 2)

                def unit(i):
                    mains(*units[i])
                    # stats lag mains by TWO units so the DVE square is
                    # never on the PE critical path
                    if len(pend) >= 2:
                        stats(*pend.pop(0))
                    pend.append(units[i])

                # both h-multiplies use repl(step-2) (fully landed); they
                # are interleaved into the DVE queue just before their
                # consumer units so they never head-of-line-block squares
                idx_l1 = next((i for i, u in enumerate(units)
                               if u[1] == 1), None)
                idx_l2 = next((i for i, u in enumerate(units)
                               if u[1] == 2), None)

                if units:
                    if idx_l1 == 0 and 0 <= step - 2 < NB:
                        apply_h(step - 2, step - 2, 0)
                    if idx_l2 == 0 and 0 <= step - 4 < NB:
                        apply_h(step - 2, step - 4, 1)
                    unit(0)
                else:
                    while pend:
                        stats(*pend.pop(0))
                if len(units) > 1:
                    if 1 == idx_l1 and 0 <= step - 2 < NB:
                        apply_h(step - 2, step - 2, 0)
                    if 1 == idx_l2 and 0 <= step - 4 < NB:
                        apply_h(step - 2, step - 4, 1)
                    unit(1)
                elif not units:
                    while pend:
                        stats(*pend.pop(0))
                # L2h1(step-5)'s stats flushed at units 0-1 -> its bank
                # drains now, freeing st2ps well before this step's L2 stats
                if 0 <= step - 5 < NB:
                    drain_l2(step - 5)
                for i in range(2, len(units)):
                    if i == idx_l1 and 0 <= step - 2 < NB:
                        apply_h(step - 2, step - 2, 0)
                    if i == idx_l2 and 0 <= step - 4 < NB:
                        apply_h(step - 2, step - 4, 1)
                    unit(i)
                # flush any pending unit whose bank drains this step (only
                # happens in warmup/tail steps; steady state keeps the lag)
                while pend and pend[0][0] + (0, 2, 5)[pend[0][1]] <= step:
                    stats(*pend.pop(0))

                if step in bankst:
                    drain_pair(step)

                for blk in sym_prep_sched.get(step, []):
                    sym_prep(blk)
                for (bi, bj) in sym_sched.get(step, []):
                    sym_pair(bi, bj)
                replst.pop(step - 3, None)

            prefetch_x(1)
            for step in range(NB + 5):
                emit_step(step)
            mstack.close()

    nc.compile()
    return nc


def _prep_weights(inputs):
    W1 = np.asarray(inputs["W1"], np.float64)
    W2 = np.asarray(inputs["W2"], np.float64)
    W3 = np.asarray(inputs["W3"], np.float64)
    Wo = np.asarray(inputs["Wo"], np.float64)
    b1 = np.asarray(inputs["b1"], np.float64)
    b2 = np.asarray(inputs["b2"], np.float64)
    b3 = np.asarray(inputs["b3"], np.float64)
    bo = np.asarray(inputs["bo"], np.float64)
    ln_g = np.asarray(inputs["ln_g"], np.float64)
    ln_b = np.asarray(inputs["ln_b"], np.float64)

    C = np.eye(H) - np.ones((H, H)) / H
    F = C @ np.diag(ln_g)
    Ws = [W1, F @ W2, F @ W3]
    bs = [b1, b2 + W2.T @ ln_b, b3 + W3.T @ ln_b]
    Woh = 0.5 * (F @ Wo)
    boh = (bo + Wo.T @ ln_b).astype(np.float32)

    wmain = np.zeros((128, 3 * 128), np.float16)
    for l, W in enumerate(Ws):
        wmain[0:64, 128 * l:128 * l + 64] = W.astype(np.float16)
        wmain[64:128, 128 * l + 64:128 * l + 128] = W.astype(np.float16)

    # per bt: g-selector cols 0:32 (rows 0:8 A-mean(bt), 8:16 B-mean),
    # s-selector cols 32:64 (rows 16:24 A-msq, 24:32 B-msq)
    wstat = np.zeros((128, SB * 64), np.float16)
    for bt in range(SB):
        wstat[0:64, 64 * bt + bt] = np.float16(1 / 64)
        wstat[64:128, 64 * bt + 8 + bt] = np.float16(1 / 64)
        wstat[0:64, 64 * bt + 32 + 16 + bt] = np.float16(1 / 64)
        wstat[64:128, 64 * bt + 32 + 24 + bt] = np.float16(1 / 64)

    # L2 merged: per bt [128, 112]: mean cols 0:32 + proj rows
    # 32 + 40g + 8v + bt
    wsp = np.zeros((128, SB * 112), np.float16)
    w16 = Woh.astype(np.float16)
    for bt in range(SB):
        wsp[0:64, 112 * bt + bt] = np.float16(1 / 64)
        wsp[64:128, 112 * bt + 8 + bt] = np.float16(1 / 64)
        for v in range(V):
            wsp[0:64, 112 * bt + 32 + 8 * v + bt] = w16[:, v]
            wsp[64:128, 112 * bt + 32 + 40 + 8 * v + bt] = w16[:, v]

    biases = np.zeros((128, 3), np.float32)
    for l, bb in enumerate(bs):
        biases[0:64, l] = bb.astype(np.float32)
        biases[64:128, l] = bb.astype(np.float32)
    id128 = np.eye(128, dtype=np.float16)
    return dict(wmain=wmain, wstat=wstat, wsp=wsp, biases=biases,
                id128=id128), boh


def _prep_x(xb):
    """[S, S, H] fp32 -> [H, T] fp16 in block-major token order."""
    t = xb.reshape(3, 128, 3, 128, H).transpose(0, 2, 1, 3, 4).reshape(T, H)
    return np.ascontiguousarray(t.T).astype(np.float16)


def kernel(**inputs):
    if "nc" not in _CACHE:
        _CACHE["nc"] = _build_nc()
    nc = _CACHE["nc"]
    weights, boh = _prep_weights(inputs)

    x = np.asarray(inputs["x"])  # [8, S, S, H] fp32
    in_maps = []
    for b in range(8):
        m = dict(weights)
        m["xf"] = _prep_x(x[b])
        in_maps.append(m)

    res = run_bass_kernel_spmd(nc, in_maps, core_ids=list(range(8)))
    outs = []
    for b in range(8):
        vm = res.results[b]["out_vm"].reshape(V, S, S).astype(np.float32)
        outs.append(vm.transpose(1, 2, 0) + boh[None, None, :])
    return np.stack(outs).astype(np.float32)


# revision 32
# speedup vs baseline: 1.2278x; 1.0097x over previous
"""Trainium2 Bass kernel for nn_BondHead2 (dense_mlp), v3.

Per batch element b (8, one per NeuronCore):
    h = LN(gelu(x @ W1 + b1)); h = LN(gelu(h @ W2 + b2)); h = LN(gelu(h @ W3 + b3))
    out = h @ Wo + bo;  out = (out + out^T_{seq axes}) / 2

v3 vs v2 (all targeting the TimelineSim cost model):
  - stats batches of 8 tiles (18 batches); the vocab projection rides the
    layer-2 mean-stats matmuls for free (stat rows 0:32 + proj rows 32:112
    share one PSUM bank; matmul cost is charged per output column only).
  - gelu / square / h-mul in [128, 2048] tiles: half the Act/DVE
    instruction count of v2.
  - L0 of batch s and L1 of batch s-2 share one stats bank (64 rows) ->
    one Act drain + one fat DMA + one rd write + one repl broadcast DMA
    per step covers both layers' rstd chains.
  - DMA consolidation: the rstd broadcast is ONE DMA per step (was 8 per
    batch-layer), mobuf write one DMA per L2 batch, fat reshape one DMA.
  - unit-level software pipeline (stats lag mains by one unit, crossing
    step boundaries) so the PE never waits on Act gelu or DVE square.
  - pipeline: L0(b)@step b, L1(b)@b+2, L2(b)@b+4, drains @b+5; both
    apply_h consumers of a step's repl run exactly 2 steps later.
"""

import numpy as np

import concourse.bacc as bacc
import concourse.bass as bass
import concourse.mybir as mybir
import concourse.tile as tile
from concourse.bass_utils import run_bass_kernel_spmd

F16 = mybir.dt.float16
F32 = mybir.dt.float32
U32 = mybir.dt.uint32
AF = mybir.ActivationFunctionType
OP = mybir.AluOpType

H = 64            # hidden dim
S = 384           # seq
T = S * S         # tokens per core (147456)
V = 5             # vocab
N = 512           # tokens per matmul tile (free dim)
SB = 8            # tiles per stats batch
NB = T // (2 * SB * N)   # 18 batches
BOFF = SB * N     # batch-local offset of group B (4096)
BLK = 2 * BOFF    # tokens per batch (8192)
SBLK = 2 * BLK    # tokens per sym block (16384; block k = batches 2k, 2k+1)
EPS = 1e-5
MAGIC = 0x5F3759DF

_CACHE: dict = {}


def _build_nc():
    nc = bacc.Bacc("TRN2", target_bir_lowering=False, debug=False)

    # ---- external inputs ----
    xf = nc.dram_tensor("xf", (H, T), F16, kind="ExternalInput").ap()
    wmain = nc.dram_tensor("wmain", (128, 3 * 128), F16, kind="ExternalInput").ap()
    # per bt: cols 0:32 g-selector (means), cols 32:64 s-selector (msqs)
    wstat = nc.dram_tensor("wstat", (128, SB * 64), F16, kind="ExternalInput").ap()
    # L2 merged: per bt [128, 112]: stats-mean cols 0:32 + proj cols 32:112
    wsp = nc.dram_tensor("wsp", (128, SB * 112), F16, kind="ExternalInput").ap()
    biases = nc.dram_tensor("biases", (128, 3), F32, kind="ExternalInput").ap()
    id128 = nc.dram_tensor("id128", (128, 128), F16, kind="ExternalInput").ap()

    # ---- internal DRAM ----
    mobuf = nc.dram_tensor("mobuf", (V * T,), F16)          # [v, tok] linear
    r3buf = nc.dram_tensor("r3buf", (T,), F16)              # [tok] linear
    rdbuf = nc.dram_tensor("rdbuf", (NB + 2, 2 * BLK), F16)  # rstd per step

    out_vm = nc.dram_tensor("out_vm", (V * T,), F16, kind="ExternalOutput").ap()

    with tile.TileContext(nc) as tc:
        with tc.tile_pool(name="wpool", bufs=1) as wpool:
            from contextlib import ExitStack
            mstack = ExitStack()
            xpool = mstack.enter_context(tc.tile_pool(name="xpool", bufs=3))
            gpool = mstack.enter_context(tc.tile_pool(name="gpool", bufs=18))
            spool = mstack.enter_context(tc.tile_pool(name="spool", bufs=4))
            fpool = mstack.enter_context(tc.tile_pool(name="fpool", bufs=2))
            rpool = mstack.enter_context(tc.tile_pool(name="rpool", bufs=3))
            mpsa = mstack.enter_context(
                tc.tile_pool(name="mpsa", bufs=1, space="PSUM"))
            mpsb = mstack.enter_context(
                tc.tile_pool(name="mpsb", bufs=1, space="PSUM"))
            stps = mstack.enter_context(
                tc.tile_pool(name="stps", bufs=2, space="PSUM"))
            st2ps = mstack.enter_context(
                tc.tile_pool(name="st2ps", bufs=1, space="PSUM"))
            syps = mstack.enter_context(
                tc.tile_pool(name="syps", bufs=1, space="PSUM"))
            sy = mstack.enter_context(tc.tile_pool(name="sypool", bufs=3))

            # resident weights (wmain first: the first mains need only wm
            # and the first x chunk, so keep the DMA queue short up front)
            wm = wpool.tile([128, 3 * 128], F16)
            nc.sync.dma_start(out=wm[:], in_=wmain)
            bcol = wpool.tile([128, 3], F32)
            nc.sync.dma_start(out=bcol[:], in_=biases)
            ws = wpool.tile([128, SB * 64], F16)
            wp2 = wpool.tile([128, SB * 112], F16)
            magic = wpool.tile([128, 1], U32)
            nc.vector.memset(magic[:], MAGIC)
            oneu = wpool.tile([128, 1], U32)
            nc.vector.memset(oneu[:], 1)
            chalf = wpool.tile([128, 1], F32)
            nc.vector.memset(chalf[:], -0.5)
            c15 = wpool.tile([128, 1], F32)
            nc.vector.memset(c15[:], 1.5)
            ceps = wpool.tile([128, 1], F32)
            nc.vector.memset(ceps[:], EPS)
            idt = wpool.tile([128, 128], F16)

            gstate = {}    # (b, layer) -> [g_h0, g_h1] tiles [128, 2048]
            xstate = {}    # b -> ("split"|"whole", tiles)
            bankst = {}    # step -> paired L0/L1 stats bank [64, N]
            bank2 = {}     # b -> L2 stats+proj bank [112, N]
            replst = {}    # step -> repl tile [128, 2*BOFF]
            sunits = {}    # (b, layer, half) -> s tile
            pend = []      # stats units pending (lag 1 behind mains)

            def rsqrt_to(v_f32, out_ap, sh, tg):
                """out <- rsqrt(v) via quake seed + 1 Newton step.

                Seed bit ops on DVE (hw GPSIMD lacks 32-bit shift); the
                Newton polynomial runs on the otherwise idle Pool engine.
                """
                y32 = fpool.tile(list(sh), F32, tag="nry" + tg)
                yi = y32[:].bitcast(U32)
                vi = v_f32.bitcast(U32)
                nc.vector.tensor_tensor(
                    yi, vi, oneu[:].to_broadcast(sh), OP.logical_shift_right)
                nc.vector.tensor_tensor(
                    yi, magic[:].to_broadcast(sh), yi, OP.subtract)
                t = fpool.tile(list(sh), F32, tag="nrt" + tg)
                nc.gpsimd.tensor_mul(t[:], y32[:], y32[:])
                nc.gpsimd.tensor_mul(t[:], t[:], v_f32)
                nc.gpsimd.tensor_mul(t[:], t[:], chalf[:].to_broadcast(sh))
                nc.gpsimd.tensor_tensor(
                    t[:], t[:], c15[:].to_broadcast(sh), OP.add)
                nc.gpsimd.tensor_mul(out_ap, t[:], y32[:])

            def prefetch_x(b, split=False):
                if split:
                    xch = []
                    for c in range(4):
                        xt = xpool.tile([128, 1024], F16, tag="xs", bufs=4)
                        src = bass.AP(
                            tensor=xf.tensor,
                            offset=b * BLK + c * 1024,
                            ap=[[BOFF, 2], [T, 64], [1, 1024]],
                        )
                        nc.sync.dma_start(out=xt[:], in_=src)
                        xch.append(xt)
                    xstate[b] = ("split", xch)
                else:
                    xt = xpool.tile([128, BOFF], F16, tag="x")
                    src = bass.AP(
                        tensor=xf.tensor,
                        offset=b * BLK,
                        ap=[[BOFF, 2], [T, 64], [1, BOFF]],
                    )
                    nc.sync.dma_start(out=xt[:], in_=src)
                    xstate[b] = ("whole", xt)

            def x_slice(b, bt):
                kind, xv = xstate[b]
                if kind == "split":
                    return xv[bt // 2][:, (bt % 2) * N:(bt % 2 + 1) * N]
                return xv[:, bt * N:(bt + 1) * N]

            # ---------------- unit machinery ----------------

            def mains(b, layer, half):
                # two alternating 2-bank PSUM tiles: pool rotation is
                # tile-granular, so each gelu only gates the SAME pool's
                # mains two half-units later (plenty of slack)
                g = gpool.tile([128, 4 * N], F16, tag="g")
                for p, pool in enumerate((mpsa, mpsb)):
                    mpair = pool.tile([128, 2, N], F32, tag="m", name="mp")
                    for k in range(2):
                        bt = 4 * half + 2 * p + k
                        if layer == 0:
                            rhs = x_slice(b, bt)
                        else:
                            gh = gstate[(b, layer - 1)][bt // 4]
                            rhs = gh[:, (bt % 4) * N:(bt % 4 + 1) * N]
                        nc.tensor.matmul(
                            mpair[:, k, :],
                            wm[:, 128 * layer:128 * (layer + 1)],
                            rhs, start=True, stop=True,
                        )
                    nc.scalar.activation(
                        g[:, 2 * N * p:2 * N * (p + 1)],
                        mpair[:].rearrange("p a n -> p (a n)"),
                        AF.Gelu, bias=bcol[:, layer:layer + 1], scale=1.0,
                    )
                if layer > 0 and half == 1:
                    gstate.pop((b, layer - 1))
                if layer == 0 and half == 1:
                    xstate.pop(b, None)
                s = spool.tile([128, 4 * N], F16, tag="s", bufs=4)
                nc.vector.tensor_mul(s[:], g[:], g[:])
                gstate.setdefault((b, layer), [None, None])[half] = g
                sunits[(b, layer, half)] = s

            def stats(b, layer, half):
                """8 matmuls: mean(+proj for L2) on g, msq on s."""
                g = gstate[(b, layer)][half]
                s = sunits.pop((b, layer, half))
                if layer == 2:
                    bank = bank2[b]
                    for k in range(4):
                        bt = 4 * half + k
                        nc.tensor.matmul(
                            bank[0:112], wp2[:, 112 * bt:112 * bt + 112],
                            g[:, k * N:(k + 1) * N],
                            start=(half == 0 and k == 0), stop=False,
                            skip_group_check=True,
                        )
                        nc.tensor.matmul(
                            bank[0:32], ws[:, 64 * bt + 32:64 * bt + 64],
                            s[:, k * N:(k + 1) * N],
                            start=False, stop=(half == 1 and k == 3),
                            skip_group_check=True,
                        )
                else:
                    # L0 of batch b shares bank[64, N] with L1 of batch b-2
                    # (same step): L0 rows 0:32, L1 rows 32:64.  Each 32-row
                    # region is its own accumulation group (start=True on
                    # its first matmul resets only that region).
                    step_key = b + 2 * layer
                    bank = bankst[step_key]
                    for k in range(4):
                        bt = 4 * half + k
                        nc.tensor.matmul(
                            bank[32 * layer:32 * layer + 32],
                            ws[:, 64 * bt:64 * bt + 32],
                            g[:, k * N:(k + 1) * N],
                            start=(half == 0 and k == 0), stop=False,
                            skip_group_check=True,
                        )
                        nc.tensor.matmul(
                            bank[32 * layer:32 * layer + 32],
                            ws[:, 64 * bt + 32:64 * bt + 64],
                            s[:, k * N:(k + 1) * N],
                            start=False, stop=(half == 1 and k == 3),
                            skip_group_check=True,
                        )

            # ---------------- stats postprocessing ----------------
            # Bank row layout per 32-row region: 0:8 grpA-mean(bt), 8:16
            # grpB-mean, 16:24 grpA-msq, 24:32 grpB-msq; bank col = c (512).
            # fat layout: partition p = 64l + 32g + 4bt + c//128, giving a
            # FLAT rf index = group-local token, so the rd write is linear.

            def drain_pair(step):
                bank = bankst.pop(step)
                rowb = fpool.tile([64, N], F16, tag="rowb")
                nc.scalar.copy(rowb[:], bank[:])
                # mean rows 32l + 8g + bt (msq rows between the l blocks)
                # -> meanfat flat index = l*BLK + g*BOFF + bt*N + c
                meanfat = fpool.tile([128, 128], F16, tag="meanfat")
                msqfat = fpool.tile([128, 128], F16, tag="msqfat")
                for l in range(2):
                    nc.sync.dma_start(
                        out=meanfat[64 * l:64 * (l + 1), :],
                        in_=rowb[32 * l:32 * l + 16, :]
                        .rearrange("p (q c) -> p q c", q=4),
                    )
                    nc.sync.dma_start(
                        out=msqfat[64 * l:64 * (l + 1), :],
                        in_=rowb[32 * l + 16:32 * l + 32, :]
                        .rearrange("p (q c) -> p q c", q=4),
                    )
                # var = msq - mean^2 + eps  (fp32, on Pool)
                sqf = fpool.tile([128, 128], F32, tag="sqf")
                nc.gpsimd.tensor_mul(sqf[:], meanfat[:], meanfat[:])
                varf = fpool.tile([128, 128], F32, tag="varf")
                nc.gpsimd.tensor_tensor(
                    varf[:], msqfat[:], sqf[:], OP.subtract)
                nc.gpsimd.tensor_tensor(
                    varf[:], varf[:], ceps[:].to_broadcast((128, 128)), OP.add)
                rf = fpool.tile([128, 128], F16, tag="rf")
                rsqrt_to(varf[:], rf[:], (128, 128), "p")
                # rf flat index == l*BLK + g*BOFF + bt*N + c  -> linear write
                nc.sync.dma_start(
                    out=bass.AP(tensor=rdbuf, offset=step * 2 * BLK,
                                ap=[[128, 128], [1, 128]]),
                    in_=rf[:],
                )
                repl = rpool.tile([128, 2 * BOFF], F16, tag="repl")
                for l in range(2):
                    src2 = bass.AP(
                        tensor=rdbuf, offset=step * 2 * BLK + l * BLK,
                        # dims (grp, dup64, c)
                        ap=[[BOFF, 2], [0, 64], [1, BOFF]],
                    )
                    nc.sync.dma_start(
                        out=repl[:, l * BOFF:(l + 1) * BOFF], in_=src2)
                replst[step] = repl

            def apply_h(step, b, layer):
                """h = g * rstd in place (repl cols: L0 0:BOFF, L1 BOFF:)."""
                repl = replst[step]
                off = layer * BOFF
                for half in range(2):
                    g = gstate[(b, layer)][half]
                    nc.vector.tensor_mul(
                        g[:], g[:],
                        repl[:, off + half * 4 * N: off + (half + 1) * 4 * N])

            def drain_l2(b):
                """L2 bank: stats rows 0:32 -> r3buf; proj rows 32:112 -> mobuf."""
                bank = bank2.pop(b)
                dr = fpool.tile([112, N], F16, tag="dr2", bufs=3)
                nc.scalar.copy(dr[:], bank[:])
                gstate.pop((b, 2), None)
                # proj rows 32:112 are (g, v, bt)-ordered: plain read, the
                # structure lives on the DRAM side: dims (g, v, bt, c)
                for g2 in range(2):
                    dst = bass.AP(
                        tensor=mobuf, offset=b * BLK + g2 * BOFF,
                        ap=[[T, V], [1, BOFF]],
                    )
                    nc.sync.dma_start(
                        out=dst, in_=dr[32 + 40 * g2:72 + 40 * g2, :])
                # stats: mean rows 0:16 / msq rows 16:32, both linear in
                # (g, bt, c) -> [128, 64] fats with flat index = token
                mean2 = fpool.tile([128, 64], F16, tag="mean2")
                nc.sync.dma_start(
                    out=mean2[:],
                    in_=dr[0:16, :].rearrange("p (q c) -> p q c", q=8),
                )
                msq2 = fpool.tile([128, 64], F16, tag="msq2")
                nc.sync.dma_start(
                    out=msq2[:],
                    in_=dr[16:32, :].rearrange("p (q c) -> p q c", q=8),
                )
                sqf = fpool.tile([128, 64], F32, tag="sqf2")
                nc.gpsimd.tensor_mul(sqf[:], mean2[:], mean2[:])
                varf = fpool.tile([128, 64], F32, tag="varf2")
                nc.gpsimd.tensor_tensor(
                    varf[:], msq2[:], sqf[:], OP.subtract)
                nc.gpsimd.tensor_tensor(
                    varf[:], varf[:], ceps[:].to_broadcast((128, 64)), OP.add)
                rf3 = fpool.tile([128, 64], F16, tag="rf3")
                rsqrt_to(varf[:], rf3[:], (128, 64), "3")
                # rf3 flat index == group-local token -> linear write
                nc.sync.dma_start(
                    out=bass.AP(tensor=r3buf, offset=b * BLK,
                                ap=[[64, 128], [1, 64]]),
                    in_=rf3[:],
                )

            # ---------------- symmetrization ----------------
            sym_pmap = {}

            def sym_prep(blk):
                mo = sy.tile([128, V, 128], F16, tag="mo_in", bufs=2)
                src = bass.AP(
                    tensor=mobuf, offset=blk * SBLK,
                    ap=[[128, 128], [T, V], [1, 128]],
                )
                nc.sync.dma_start(out=mo[:], in_=src)
                r = sy.tile([128, 128], F16, tag="r_in", bufs=2)
                rsrc = bass.AP(
                    tensor=r3buf, offset=blk * SBLK,
                    ap=[[128, 128], [1, 128]],
                )
                nc.sync.dma_start(out=r[:], in_=rsrc)
                p_ = sy.tile([128, V, 128], F16, tag="p", bufs=5)
                ra = r[:]
                rbc = bass.AP(tensor=ra.tensor, offset=ra.offset,
                              ap=[ra.ap[0], [0, V], ra.ap[1]])
                # on Pool: its DMA-wait never blocks the DVE square queue
                nc.gpsimd.tensor_mul(p_[:], mo[:], rbc)
                sym_pmap[blk] = p_

            def sym_transposes(p_):
                pt = syps.tile([128, V, 128], F16, tag="pt")
                for v in range(V):
                    nc.tensor.transpose(pt[:, v, :], p_[:, v, :], idt[:])
                return pt

            def sym_emit(pa, pt, bi, bj):
                o = sy.tile([128, V, 128], F16, tag="o", bufs=2)
                nc.vector.tensor_add(
                    o[:].rearrange("p a n -> p (a n)"),
                    pa[:].rearrange("p a n -> p (a n)"),
                    pt[:].rearrange("p a n -> p (a n)"),
                )
                d1 = bass.AP(
                    tensor=out_vm.tensor, offset=bi * 128 * S + bj * 128,
                    ap=[[S, 128], [T, V], [1, 128]],
                )
                nc.sync.dma_start(out=d1, in_=o[:])

            def sym_pair(bi, bj):
                pa = sym_pmap.pop(3 * bi + bj)
                if bi == bj:
                    sym_emit(pa, sym_transposes(pa), bi, bj)
                else:
                    pb = sym_pmap.pop(3 * bj + bi)
                    sym_emit(pa, sym_transposes(pb), bi, bj)
                    sym_emit(pb, sym_transposes(pa), bj, bi)

            # sym block k = batches 2k, 2k+1; r3/mobuf of batch b land in
            # step b+5, so block k preps at step 2k+6 and a pair fires once
            # both its blocks are prepped.
            sym_sched = {}
            sym_prep_sched = {}
            for blk in range(9):
                sym_prep_sched.setdefault(2 * blk + 6, []).append(blk)
            for bi in range(3):
                for bj in range(bi + 1):
                    gate = 2 * max(3 * bi + bj, 3 * bj + bi) + 6
                    sym_sched.setdefault(gate, []).append((bi, bj))

            prefetch_x(0, split=True)
            nc.sync.dma_start(out=ws[:], in_=wstat)
            nc.sync.dma_start(out=wp2[:], in_=wsp)
            nc.sync.dma_start(out=idt[:], in_=id128)

            # pipeline: L0(b)@b, L1(b)@b+2, L2(b)@b+3; drain_l2(b)@b+4
            # (early in the step, so the freed bank never stalls this
            # step's L2 stats); stats lag mains by one unit crossing step
            # boundaries; apply_h(L0(b)) runs in step b+1's DVE tail,
            # apply_h(L1(b)) early in step b+3 (interleaved after unit 1
            # so it never heads-of-line-blocks the squares).
            def emit_step(step):
                units = []
                if step < NB:
                    units.append((step, 0, 0))
                    units.append((step, 0, 1))
                if 0 <= step - 2 < NB:
                    units.append((step - 2, 1, 0))
                    units.append((step - 2, 1, 1))
                if 0 <= step - 4 < NB:
                    units.append((step - 4, 2, 0))
                    units.append((step - 4, 2, 1))

                has_l0 = step < NB
                has_l1 = 0 <= step - 2 < NB
                if has_l0 or has_l1:
                    bk = stps.tile([64, N], F32, tag="stat", name="bk")
                    bankst[step] = bk
                    if not has_l0:
                        nc.vector.memset(bk[0:32], 0.0)
                    if not has_l1:
                        nc.vector.memset(bk[32:64], 0.0)
                if 0 <= step - 4 < NB:
                    bank2[step - 4] = st2ps.tile([112, N], F32, tag="st2",
                                                 name="bk2")

                if step + 2 < NB:
                    prefetch_x(step + 2)

                def unit(i):
                    mains(*units[i])
                    # stats lag mains by TWO units so the DVE square is
                    # never on the PE critical path
                    if len(pend) >= 2:
                        stats(*pend.pop(0))
                    pend.append(units[i])

                # both h-multiplies use repl(step-2) (fully landed); they
                # are interleaved into the DVE queue just before their
                # consumer units so they never head-of-line-block squares
                idx_l1 = next((i for i, u in enumerate(units)
                               if u[1] == 1), None)
                idx_l2 = next((i for i, u in enumerate(units)
                               if u[1] == 2), None)

                if units:
                    if idx_l1 == 0 and 0 <= step - 2 < NB:
                        apply_h(step - 2, step - 2, 0)
                    if idx_l2 == 0 and 0 <= step - 4 < NB:
                        apply_h(step - 2, step - 4, 1)
                    unit(0)
                else:
                    while pend:
                        stats(*pend.pop(0))
                if len(units) > 1:
                    if 1 == idx_l1 and 0 <= step - 2 < NB:
                        apply_h(step - 2, step - 2, 0)
                    if 1 == idx_l2 and 0 <= step - 4 < NB:
                        apply_h(step - 2, step - 4, 1)
                    unit(1)
                elif not units:
                    while pend:
                        stats(*pend.pop(0))
                # L2h1(step-5)'s stats flushed at units 0-1 -> its bank
                # drains now, freeing st2ps well before this step's L2 stats
                if 0 <= step - 5 < NB:
                    drain_l2(step - 5)
                for i in range(2, len(units)):
                    if i == idx_l1 and 0 <= step - 2 < NB:
                        apply_h(step - 2, step - 2, 0)
                    if i == idx_l2 and 0 <= step - 4 < NB:
                        apply_h(step - 2, step - 4, 1)
                    unit(i)
                # flush any pending unit whose bank drains this step (only
                # happens in warmup/tail steps; steady state keeps the lag)
                while pend and pend[0][0] + (0, 2, 5)[pend[0][1]] <= step:
                    stats(*pend.pop(0))

                if step in bankst:
                    drain_pair(step)

                for blk in sym_prep_sched.get(step, []):
                    sym_prep(blk)
                for (bi, bj) in sym_sched.get(step, []):
                    sym_pair(bi, bj)
                replst.pop(step - 3, None)

            prefetch_x(1)
            for step in range(NB + 5):
                emit_step(step)
            mstack.close()

    nc.compile()
    return nc


def _prep_weights(inputs):
    W1 = np.asarray(inputs["W1"], np.float64)
    W2 = np.asarray(inputs["W2"], np.float64)
    W3 = np.asarray(inputs["W3"], np.float64)
    Wo = np.asarray(inputs["Wo"], np.float64)
    b1 = np.asarray(inputs["b1"], np.float64)
    b2 = np.asarray(inputs["b2"], np.float64)
    b3 = np.asarray(inputs["b3"], np.float64)
    bo = np.asarray(inputs["bo"], np.float64)
    ln_g = np.asarray(inputs["ln_g"], np.float64)
    ln_b = np.asarray(inputs["ln_b"], np.float64)

    C = np.eye(H) - np.ones((H, H)) / H
    F = C @ np.diag(ln_g)
    Ws = [W1, F @ W2, F @ W3]
    bs = [b1, b2 + W2.T @ ln_b, b3 + W3.T @ ln_b]
    Woh = 0.5 * (F @ Wo)
    boh = (bo + Wo.T @ ln_b).astype(np.float32)

    wmain = np.zeros((128, 3 * 128), np.float16)
    for l, W in enumerate(Ws):
        wmain[0:64, 128 * l:128 * l + 64] = W.astype(np.float16)
        wmain[64:128, 128 * l + 64:128 * l + 128] = W.astype(np.float16)

    # per bt: g-selector cols 0:32 (rows 0:8 A-mean(bt), 8:16 B-mean),
    # s-selector cols 32:64 (rows 16:24 A-msq, 24:32 B-msq)
    wstat = np.zeros((128, SB * 64), np.float16)
    for bt in range(SB):
        wstat[0:64, 64 * bt + bt] = np.float16(1 / 64)
        wstat[64:128, 64 * bt + 8 + bt] = np.float16(1 / 64)
        wstat[0:64, 64 * bt + 32 + 16 + bt] = np.float16(1 / 64)
        wstat[64:128, 64 * bt + 32 + 24 + bt] = np.float16(1 / 64)

    # L2 merged: per bt [128, 112]: mean cols 0:32 + proj rows
    # 32 + 40g + 8v + bt
    wsp = np.zeros((128, SB * 112), np.float16)
    w16 = Woh.astype(np.float16)
    for bt in range(SB):
        wsp[0:64, 112 * bt + bt] = np.float16(1 / 64)
        wsp[64:128, 112 * bt + 8 + bt] = np.float16(1 / 64)
        for v in range(V):
            wsp[0:64, 112 * bt + 32 + 8 * v + bt] = w16[:, v]
            wsp[64:128, 112 * bt + 32 + 40 + 8 * v + bt] = w16[:, v]

    biases = np.zeros((128, 3), np.float32)
    for l, bb in enumerate(bs):
        biases[0:64, l] = bb.astype(np.float32)
        biases[64:128, l] = bb.astype(np.float32)
    id128 = np.eye(128, dtype=np.float16)
    return dict(wmain=wmain, wstat=wstat, wsp=wsp, biases=biases,
                id128=id128), boh


def _prep_x(xb):
    """[S, S, H] fp32 -> [H, T] fp16 in block-major token order."""
    t = xb.reshape(3, 128, 3, 128, H).transpose(0, 2, 1, 3, 4).reshape(T, H)
    return np.ascontiguousarray(t.T).astype(np.float16)


def kernel(**inputs):
    if "nc" not in _CACHE:
        _CACHE["nc"] = _build_nc()
    nc = _CACHE["nc"]
    weights, boh = _prep_weights(inputs)

    x = np.asarray(inputs["x"])  # [8, S, S, H] fp32
    in_maps = []
    for b in range(8):
        m = dict(weights)
        m["xf"] = _prep_x(x[b])
        in_maps.append(m)

    res = run_bass_kernel_spmd(nc, in_maps, core_ids=list(range(8)))
    outs = []
    for b in range(8):
        vm = res.results[b]["out_vm"].reshape(V, S, S).astype(np.float32)
        outs.append(vm.transpose(1, 2, 0) + boh[None, None, :])
    return np.stack(outs).astype(np.float32)
